# revision 13
# baseline (speedup 1.0000x reference)
"""TRN2 Bass/Tile kernel for nn_BBoxHead (2000 ROIs x {GEMM 12544->1024, BN, ReLU,
GEMM 1024->1024, BN, ReLU, logits/softmax + box-delta heads}).

Strategy: data-parallel over the ROI axis (250 ROIs per core on 8 NeuronCores),
weights replicated, fp16 storage + matmul with fp32 PSUM accumulation. BN is
training-mode (batch stats over all 2000 ROIs), so per-core partial sums/sumsqs
are combined with tiny AllReduces. GEMM1 is split into two 512-channel halves so
the first AllReduce overlaps the second half's matmuls, and a dummy AllReduce at
kernel start absorbs inter-core launch skew + the collective entry barrier while
the first GEMM runs.

Layout on chip: activations are [channels(partitions), rois(free)], so BN stats
are free-axis reductions and BN apply is a per-partition affine + ReLU in a
single scalar-engine activation. The head GEMMs use the activations as the
stationary operand, producing [rois(partitions), classes(free)] so softmax is a
free-axis reduction too.

All DRAM inputs are host-prepacked so every DMA is per-partition contiguous
(large descriptors): weight k-tile t, partition p holds row k=128t+p.
"""
import numpy as np

import concourse.bass as bass
import concourse.mybir as mybir
import concourse.tile as tile
from concourse import bacc
from concourse.bass_utils import run_bass_kernel_spmd

N_CORES = 8
N_ROI = 2000
NROI_C = N_ROI // N_CORES  # 250
K1 = 7 * 7 * 256  # 12544
K1T = K1 // 128  # 98
H = 1024
HT = H // 128  # 8
NCLS = 81
ND = NCLS * 4  # 324
EPS = 1e-3
# k-tile chunking of the w1/rois streams: a small first chunk so the PE can
# start early, 14-tile chunks after (14KB/partition contiguous per DMA).
CH = [4] + [14] * 6 + [10]  # sums to 98
CH_OFF = [sum(CH[:i]) for i in range(len(CH))]
NRCH = len(CH)
K2CH = [(i, k - CH_OFF[i]) for i in range(NRCH) for k in range(CH_OFF[i], CH_OFF[i] + CH[i])]
F16 = mybir.dt.float16
F32 = mybir.dt.float32
AX = mybir.AxisListType.X
AF = mybir.ActivationFunctionType


def _bcast_ap(handle, parts=128):
    """DRAM [n] -> broadcast AP [[0, parts], [1, n]] (same row to all partitions)."""
    ap = handle.ap()
    return bass.AP(tensor=ap.tensor, offset=ap.offset, ap=[[0, parts], *ap.ap])


def build():
    nc = bacc.Bacc("TRN2", target_bir_lowering=False, debug=False, num_devices=N_CORES)

    rois_d = nc.dram_tensor("rois", [128, K1T, NROI_C], F16, kind="ExternalInput")
    w1a_d = nc.dram_tensor("w1a", [128, K1T, 512], F16, kind="ExternalInput")
    w1b_d = nc.dram_tensor("w1b", [128, K1T, 512], F16, kind="ExternalInput")
    w2_d = nc.dram_tensor("w2", [128, HT, H], F16, kind="ExternalInput")
    w3_d = nc.dram_tensor("w3", [128, HT, NCLS], F16, kind="ExternalInput")
    w4_d = nc.dram_tensor("w4", [128, HT, ND], F16, kind="ExternalInput")
    g1_d = nc.dram_tensor("g1", [128, HT], F32, kind="ExternalInput")
    b1_d = nc.dram_tensor("b1", [128, HT], F32, kind="ExternalInput")
    g2_d = nc.dram_tensor("g2", [128, HT], F32, kind="ExternalInput")
    b2_d = nc.dram_tensor("b2", [128, HT], F32, kind="ExternalInput")
    lb_d = nc.dram_tensor("lb", [NCLS], F32, kind="ExternalInput")
    db_d = nc.dram_tensor("db", [ND], F32, kind="ExternalInput")

    logits_d = nc.dram_tensor("logits", [NROI_C, NCLS], F32, kind="ExternalOutput")
    probs_d = nc.dram_tensor("probs", [NROI_C, NCLS], F32, kind="ExternalOutput")
    deltas_d = nc.dram_tensor("deltas", [NROI_C, ND], F32, kind="ExternalOutput")

    w1half_d = [w1a_d, w1b_d]

    with tile.TileContext(nc) as tc:
        with (
            tc.tile_pool(name="persist", bufs=1) as pp,
            tc.tile_pool(name="w1s", bufs=3) as w1p,
            tc.tile_pool(name="scratch", bufs=2) as sp,
            tc.tile_pool(name="psum", bufs=8, space="PSUM") as psp,
            tc.tile_pool(name="dramp", bufs=1, space="DRAM") as dp,
        ):
            # ---- early sync AllReduce: absorbs launch skew + collective entry
            # barrier while GEMM1 runs. It carries EPS/8 per core (sum = EPS
            # exactly: scaling by 8 is exponent-only), and its output is the
            # eps bias of the BN Sqrt — whose real dependency (the stats
            # AllReduce) always completes later, so nothing serializes on it.
            esrc = pp.tile([128, 1], F32, tag="esrc", name="esrc")
            nc.vector.memset(esrc[:], EPS / N_CORES)
            ccin_s = dp.tile([128, 1], F32, tag="ccin_s", name="ccin_s")
            ccout_s = dp.tile([128, 1], F32, tag="ccout_s", name="ccout_s")
            nc.gpsimd.dma_start(ccin_s[:], esrc[:])
            nc.gpsimd.collective_compute(
                "AllReduce",
                mybir.AluOpType.add,
                replica_groups=[list(range(N_CORES))],
                ins=[ccin_s[:].opt()],
                outs=[ccout_s[:].opt()],
            )
            eps_sb = pp.tile([128, 1], F32, tag="epssb", name="epssb")
            nc.gpsimd.dma_start(eps_sb[:], ccout_s[:])

            zero_sb = pp.tile([128, 1], F32, tag="zerosb", name="zerosb")
            nc.vector.memset(zero_sb[:], 0.0)

            # ---- small constants via GpSimd DGE (keeps the Sync queue free
            # for the rois/w1 stream) ----
            g1_sb = pp.tile([128, HT], F32, tag="g1sb", name="g1sb")
            nc.gpsimd.dma_start(g1_sb[:], g1_d[:, :])
            b1_sb = pp.tile([128, HT], F32, tag="b1sb", name="b1sb")
            nc.gpsimd.dma_start(b1_sb[:], b1_d[:, :])
            g2_sb = pp.tile([128, HT], F32, tag="g2sb", name="g2sb")
            nc.gpsimd.dma_start(g2_sb[:], g2_d[:, :])
            b2_sb = pp.tile([128, HT], F32, tag="b2sb", name="b2sb")
            nc.gpsimd.dma_start(b2_sb[:], b2_d[:, :])
            lb_sb = pp.tile([128, NCLS], F32, tag="lbsb", name="lbsb")
            nc.gpsimd.dma_start(lb_sb[:], _bcast_ap(lb_d))
            db_sb = pp.tile([128, ND], F32, tag="dbsb", name="dbsb")
            nc.gpsimd.dma_start(db_sb[:], _bcast_ap(db_d))

            # ---- rois resident; streamed interleaved with w1 half-A chunks.
            # Two HWDGE rings (Sync + Scalar) carry the streams in parallel:
            # rois + 2 w1a chunks on Sync (~9.6MB), the rest of w1a on Scalar
            # (~9.5MB), so neither ring is the bottleneck.
            rois_sb = []
            for i in range(NRCH):
                t = pp.tile([128, CH[i], NROI_C], F16, tag=f"rois{i}", name=f"rois{i}")
                rois_sb.append(t)

            w1t = [[None] * NRCH, [None] * NRCH]

            def load_w1_chunk(h, i, eng):
                t = w1p.tile([128, CH[i], 512], F16, tag="w1t", name=f"w1t_{h}_{i}")
                eng.dma_start(t[:], w1half_d[h][:, CH_OFF[i]:CH_OFF[i] + CH[i], :])
                w1t[h][i] = t

            for i in range(NRCH):
                nc.sync.dma_start(
                    rois_sb[i][:], rois_d[:, CH_OFF[i]:CH_OFF[i] + CH[i], :]
                )
                load_w1_chunk(0, i, nc.sync if i in (3, 6) else nc.scalar)

            # ---- GEMM1 x1[m(chan), n(roi)] += w1[k, m].T @ roisT[k, n] ----
            x1ps = []
            for m in range(HT):
                x1ps.append(psp.tile([128, NROI_C], F32, tag="ps", name=f"x1ps{m}"))
            x1n = []  # bn1+relu output, fp16
            for m in range(HT):
                x1n.append(pp.tile([128, NROI_C], F16, tag=f"x1n{m}", name=f"x1n{m}"))

            statsg = [None, None]  # AllReduced [sum(4) | sumsq(4)] per half

            def gemm1_half(h):
                for k in range(K1T):
                    ci, co = K2CH[k]
                    wt = w1t[h][ci][:, co, :]
                    rch = rois_sb[ci][:, co, :]
                    for mi in range(4):
                        nc.tensor.matmul(
                            x1ps[4 * h + mi][:],
                            wt[:, mi * 128:(mi + 1) * 128],
                            rch,
                            start=(k == 0),
                            stop=(k == K1T - 1),
                        )
                # local stats: cols 0..3 = sum, 4..7 = sumsq
                stats = pp.tile([128, 8], F32, tag=f"stats{h}", name=f"stats{h}")
                for mi in range(4):
                    nc.vector.reduce_sum(
                        stats[:, mi:mi + 1], x1ps[4 * h + mi][:], axis=AX
                    )
                    sq = sp.tile([128, NROI_C], F32, tag="sqscr", name=f"sq{h}{mi}")
                    nc.scalar.activation(
                        sq[:], x1ps[4 * h + mi][:], AF.Square,
                        bias=zero_sb[:, 0:1],
                        accum_out=stats[:, 4 + mi:5 + mi],
                    )
                ccin = dp.tile([128, 8], F32, tag=f"ccin{h}", name=f"ccin{h}")
                ccout = dp.tile([128, 8], F32, tag=f"ccout{h}", name=f"ccout{h}")
                nc.sync.dma_start(ccin[:], stats[:])
                nc.gpsimd.collective_compute(
                    "AllReduce",
                    mybir.AluOpType.add,
                    replica_groups=[list(range(N_CORES))],
                    ins=[ccin[:].opt()],
                    outs=[ccout[:].opt()],
                )
                sg = pp.tile([128, 8], F32, tag=f"statsg{h}", name=f"statsg{h}")
                nc.sync.dma_start(sg[:], ccout[:])
                statsg[h] = sg

            def bn_affine(sg, nt, gamma, beta, tag):
                """From AllReduced [sum(nt) | sumsq(nt)] compute per-channel
                scale A = gamma*rstd and shift B = beta - mean*A."""
                ms = pp.tile([128, 2 * nt], F32, tag=f"ms{tag}", name=f"ms{tag}")
                nc.vector.tensor_scalar_mul(ms[:], sg[:], 1.0 / N_ROI)
                mean = ms[:, 0:nt]
                e2 = ms[:, nt:2 * nt]
                msq = pp.tile([128, nt], F32, tag=f"msq{tag}", name=f"msq{tag}")
                nc.vector.tensor_mul(msq[:], mean, mean)
                var = pp.tile([128, nt], F32, tag=f"var{tag}", name=f"var{tag}")
                nc.vector.tensor_sub(var[:], e2, msq[:])
                sd = pp.tile([128, nt], F32, tag=f"sd{tag}", name=f"sd{tag}")
                nc.scalar.activation(sd[:], var[:], AF.Sqrt, bias=eps_sb[:, 0:1])
                rstd = pp.tile([128, nt], F32, tag=f"rstd{tag}", name=f"rstd{tag}")
                nc.vector.reciprocal(rstd[:], sd[:])
                A = pp.tile([128, nt], F32, tag=f"A{tag}", name=f"A{tag}")
                nc.vector.tensor_mul(A[:], rstd[:], gamma)
                mA = pp.tile([128, nt], F32, tag=f"mA{tag}", name=f"mA{tag}")
                nc.vector.tensor_mul(mA[:], mean, A[:])
                B = pp.tile([128, nt], F32, tag=f"B{tag}", name=f"B{tag}")
                nc.vector.tensor_sub(B[:], beta, mA[:])
                return A, B

            # half A + its stats/AllReduce
            gemm1_half(0)

            # late preloads for the second phase (emitted after the half-A
            # stream so they don't delay it; all are single large DMAs)
            w2_sb = pp.tile([128, HT, H], F16, tag="w2sb", name="w2sb")
            nc.sync.dma_start(w2_sb[:], w2_d[:, :, :])
            w3_sb = pp.tile([128, HT, NCLS], F16, tag="w3sb", name="w3sb")
            nc.sync.dma_start(w3_sb[:], w3_d[:, :, :])
            w4_sb = pp.tile([128, HT, ND], F16, tag="w4sb", name="w4sb")
            nc.sync.dma_start(w4_sb[:], w4_d[:, :, :])

            # half B: w1b chunks alternate between the two HWDGE rings
            for i in range(NRCH):
                load_w1_chunk(1, i, nc.sync if i % 2 == 0 else nc.scalar)
            gemm1_half(1)

            # bn1 apply for half A, then a head start on GEMM2 with k2 in 0..3
            A0, B0 = bn_affine(statsg[0], 4, g1_sb[:, 0:4], b1_sb[:, 0:4], "h0")
            for mi in range(4):
                nc.scalar.activation(
                    x1n[mi][:], x1ps[mi][:], AF.Relu,
                    bias=B0[:, mi:mi + 1], scale=A0[:, mi:mi + 1],
                )

            # ---- GEMM2 x2[m2, n] += w2[k2, m2].T @ x1n[k2, n] ----
            x2ps = []
            for m2 in range(4):
                x2ps.append(psp.tile([128, NROI_C], F32, tag="ps", name=f"x2ps{m2}"))
                for k2 in range(4):
                    nc.tensor.matmul(
                        x2ps[m2][:],
                        w2_sb[:, k2, m2 * 128:(m2 + 1) * 128],
                        x1n[k2][:],
                        start=(k2 == 0),
                        stop=False,
                    )

            # bn1 apply for half B
            A1, B1 = bn_affine(statsg[1], 4, g1_sb[:, 4:8], b1_sb[:, 4:8], "h1")
            for mi in range(4):
                nc.scalar.activation(
                    x1n[4 + mi][:], x1ps[4 + mi][:], AF.Relu,
                    bias=B1[:, mi:mi + 1], scale=A1[:, mi:mi + 1],
                )

            # finish GEMM2 + stats2
            stats2 = pp.tile([128, 16], F32, tag="stats2", name="stats2")

            def stats2_for(m2):
                nc.vector.reduce_sum(stats2[:, m2:m2 + 1], x2ps[m2][:], axis=AX)
                sq = sp.tile([128, NROI_C], F32, tag="sqscr", name=f"sq2_{m2}")
                nc.scalar.activation(
                    sq[:], x2ps[m2][:], AF.Square,
                    bias=zero_sb[:, 0:1],
                    accum_out=stats2[:, 8 + m2:9 + m2],
                )

            for m2 in range(4):
                for k2 in range(4, 8):
                    nc.tensor.matmul(
                        x2ps[m2][:],
                        w2_sb[:, k2, m2 * 128:(m2 + 1) * 128],
                        x1n[k2][:],
                        start=False,
                        stop=(k2 == 7),
                    )
                stats2_for(m2)
            for m2 in range(4, 8):
                x2ps.append(psp.tile([128, NROI_C], F32, tag="ps", name=f"x2ps{m2}"))
                for k2 in range(HT):
                    nc.tensor.matmul(
                        x2ps[m2][:],
                        w2_sb[:, k2, m2 * 128:(m2 + 1) * 128],
                        x1n[k2][:],
                        start=(k2 == 0),
                        stop=(k2 == 7),
                    )
                stats2_for(m2)

            ccin2 = dp.tile([128, 16], F32, tag="ccin2", name="ccin2")
            ccout2 = dp.tile([128, 16], F32, tag="ccout2", name="ccout2")
            nc.sync.dma_start(ccin2[:], stats2[:])
            nc.gpsimd.collective_compute(
                "AllReduce",
                mybir.AluOpType.add,
                replica_groups=[list(range(N_CORES))],
                ins=[ccin2[:].opt()],
                outs=[ccout2[:].opt()],
            )
            statsg2 = pp.tile([128, 16], F32, tag="statsg2", name="statsg2")
            nc.sync.dma_start(statsg2[:], ccout2[:])

            A2, B2 = bn_affine(statsg2, 8, g2_sb[:], b2_sb[:], "l2")
            x2n = []
            for m2 in range(HT):
                t = pp.tile([128, NROI_C], F16, tag=f"x2n{m2}", name=f"x2n{m2}")
                nc.scalar.activation(
                    t[:], x2ps[m2][:], AF.Relu,
                    bias=B2[:, m2:m2 + 1], scale=A2[:, m2:m2 + 1],
                )
                x2n.append(t)

            # ---- heads: out[n, c] += x2n[k, n].T @ w[k, c]; softmax over c ----
            for j, (p0, psz) in enumerate([(0, 128), (128, NROI_C - 128)]):
                pl = psp.tile([128, NCLS], F32, tag="ps", name=f"pl{j}")
                pd = psp.tile([128, ND], F32, tag="ps", name=f"pd{j}")
                for k in range(HT):
                    lhs = x2n[k][:, p0:p0 + psz]
                    nc.tensor.matmul(
                        pl[:psz, :], lhs, w3_sb[:, k, :],
                        start=(k == 0), stop=(k == HT - 1),
                    )
                    nc.tensor.matmul(
                        pd[:psz, :], lhs, w4_sb[:, k, :],
                        start=(k == 0), stop=(k == HT - 1),
                    )
                lg = sp.tile([128, NCLS], F32, tag=f"lg{j}", name=f"lg{j}")
                nc.vector.tensor_add(lg[:psz, :], pl[:psz, :], lb_sb[:psz, :])
                nc.sync.dma_start(logits_d[p0:p0 + psz, :], lg[:psz, :])

                negmx = sp.tile([128, 1], F32, tag=f"nm{j}", name=f"nm{j}")
                nc.vector.reduce_max(negmx[:psz, :], lg[:psz, :], axis=AX, negate=True)
                ex = sp.tile([128, NCLS], F32, tag=f"ex{j}", name=f"ex{j}")
                sume = sp.tile([128, 1], F32, tag=f"se{j}", name=f"se{j}")
                nc.scalar.activation(
                    ex[:psz, :], lg[:psz, :], AF.Exp,
                    bias=negmx[:psz, 0:1], accum_out=sume[:psz, 0:1],
                )
                rcp = sp.tile([128, 1], F32, tag=f"rc{j}", name=f"rc{j}")
                nc.vector.reciprocal(rcp[:psz, :], sume[:psz, :])
                pr = sp.tile([128, NCLS], F32, tag=f"pr{j}", name=f"pr{j}")
                nc.vector.tensor_scalar_mul(pr[:psz, :], ex[:psz, :], rcp[:psz, 0:1])
                nc.sync.dma_start(probs_d[p0:p0 + psz, :], pr[:psz, :])

                dl = sp.tile([128, ND], F32, tag=f"dl{j}", name=f"dl{j}")
                nc.vector.tensor_add(dl[:psz, :], pd[:psz, :], db_sb[:psz, :])
                nc.sync.dma_start(deltas_d[p0:p0 + psz, :], dl[:psz, :])

    nc.compile()
    return nc


_NC_CACHE = None


def _compiled():
    global _NC_CACHE
    if _NC_CACHE is None:
        _NC_CACHE = build()
    return _NC_CACHE


def make_in_maps(inputs):
    """Host-side shard + pack. inputs: dict of full-size float32 arrays."""
    g = {k: np.asarray(v) for k, v in inputs.items()}
    r16 = g["pooled_rois"].reshape(N_ROI, K1).astype(np.float16)
    w1t = g["conv1_w"].reshape(K1, H).astype(np.float16)
    w1t = w1t.reshape(K1T, 128, H).transpose(1, 0, 2)  # [128, 98, 1024]
    w1a = np.ascontiguousarray(w1t[:, :, :512])
    w1b = np.ascontiguousarray(w1t[:, :, 512:])
    w2 = np.ascontiguousarray(
        g["conv2_w"].reshape(H, H).astype(np.float16).reshape(HT, 128, H)
        .transpose(1, 0, 2)
    )
    w3 = np.ascontiguousarray(
        g["logits_w"].astype(np.float16).reshape(HT, 128, NCLS).transpose(1, 0, 2)
    )
    w4 = np.ascontiguousarray(
        g["delta_w"].astype(np.float16).reshape(HT, 128, ND).transpose(1, 0, 2)
    )
    g1 = np.ascontiguousarray(g["bn1_gamma"].astype(np.float32).reshape(HT, 128).T)
    b1 = np.ascontiguousarray(g["bn1_beta"].astype(np.float32).reshape(HT, 128).T)
    g2 = np.ascontiguousarray(g["bn2_gamma"].astype(np.float32).reshape(HT, 128).T)
    b2 = np.ascontiguousarray(g["bn2_beta"].astype(np.float32).reshape(HT, 128).T)
    lb = np.ascontiguousarray(g["logits_b"].astype(np.float32))
    db = np.ascontiguousarray(g["delta_b"].astype(np.float32))
    in_maps = []
    for c in range(N_CORES):
        # [250, 12544] -> roisT [12544, 250] -> [128(p), 98(k), 250(n)]
        shard = r16[c * NROI_C:(c + 1) * NROI_C, :].T
        packed = np.ascontiguousarray(
            shard.reshape(K1T, 128, NROI_C).transpose(1, 0, 2)
        )
        in_maps.append({
            "rois": packed, "w1a": w1a, "w1b": w1b, "w2": w2, "w3": w3, "w4": w4,
            "g1": g1, "b1": b1, "g2": g2, "b2": b2, "lb": lb, "db": db,
        })
    return in_maps


def run_on_cores(inputs, trace=False, tmpdir=None, **kw):
    nc = _compiled()
    in_maps = make_in_maps(inputs)
    res = run_bass_kernel_spmd(
        nc, in_maps, core_ids=list(range(N_CORES)), trace=trace, tmpdir=tmpdir, **kw
    )
    logits = np.concatenate([res.results[c]["logits"] for c in range(N_CORES)], axis=0)
    probs = np.concatenate([res.results[c]["probs"] for c in range(N_CORES)], axis=0)
    deltas = np.concatenate([res.results[c]["deltas"] for c in range(N_CORES)], axis=0)
    out = (
        logits.astype(np.float32),
        probs.astype(np.float32),
        deltas.reshape(N_ROI, NCLS, 4).astype(np.float32),
    )
    return out, res


def kernel(**inputs):
    out, _ = run_on_cores(inputs)
    return out


# revision 16
# speedup vs baseline: 1.0110x; 1.0110x over previous
"""TRN2 Bass/Tile kernel for nn_BBoxHead (2000 ROIs x {GEMM 12544->1024, BN, ReLU,
GEMM 1024->1024, BN, ReLU, logits/softmax + box-delta heads}).

Strategy: data-parallel over the ROI axis (250 ROIs per core on 8 NeuronCores),
weights replicated, fp16 storage + matmul with fp32 PSUM accumulation. BN is
training-mode (batch stats over all 2000 ROIs), so per-core partial sums/sumsqs
are combined with tiny AllReduces. GEMM1 is split into two 512-channel halves so
the first AllReduce overlaps the second half's matmuls, and a dummy AllReduce at
kernel start absorbs inter-core launch skew + the collective entry barrier while
the first GEMM runs.

Layout on chip: activations are [channels(partitions), rois(free)], so BN stats
are free-axis reductions and BN apply is a per-partition affine + ReLU in a
single scalar-engine activation. The head GEMMs use the activations as the
stationary operand, producing [rois(partitions), classes(free)] so softmax is a
free-axis reduction too.

All DRAM inputs are host-prepacked so every DMA is per-partition contiguous
(large descriptors): weight k-tile t, partition p holds row k=128t+p.
"""
import numpy as np

import concourse.bass as bass
import concourse.mybir as mybir
import concourse.tile as tile
from concourse import bacc
from concourse.bass_utils import run_bass_kernel_spmd

N_CORES = 8
N_ROI = 2000
NROI_C = N_ROI // N_CORES  # 250
K1 = 7 * 7 * 256  # 12544
K1T = K1 // 128  # 98
H = 1024
HT = H // 128  # 8
NCLS = 81
ND = NCLS * 4  # 324
EPS = 1e-3
# k-tile chunking of the w1/rois streams: a small first chunk so the PE can
# start early, 14-tile chunks after (14KB/partition contiguous per DMA).
CH = [4] + [14] * 6 + [10]  # sums to 98
CH_OFF = [sum(CH[:i]) for i in range(len(CH))]
NRCH = len(CH)
K2CH = [(i, k - CH_OFF[i]) for i in range(NRCH) for k in range(CH_OFF[i], CH_OFF[i] + CH[i])]
F16 = mybir.dt.float16
F32 = mybir.dt.float32
AX = mybir.AxisListType.X
AF = mybir.ActivationFunctionType


def _bcast_ap(handle, parts=128):
    """DRAM [n] -> broadcast AP [[0, parts], [1, n]] (same row to all partitions)."""
    ap = handle.ap()
    return bass.AP(tensor=ap.tensor, offset=ap.offset, ap=[[0, parts], *ap.ap])


def build():
    nc = bacc.Bacc("TRN2", target_bir_lowering=False, debug=False, num_devices=N_CORES)

    rois_d = nc.dram_tensor("rois", [128, K1T, NROI_C], F16, kind="ExternalInput")
    w1a_d = nc.dram_tensor("w1a", [128, K1T, 512], F16, kind="ExternalInput")
    w1b_d = nc.dram_tensor("w1b", [128, K1T, 512], F16, kind="ExternalInput")
    w2_d = nc.dram_tensor("w2", [128, HT, H], F16, kind="ExternalInput")
    w3_d = nc.dram_tensor("w3", [128, HT, NCLS], F16, kind="ExternalInput")
    w4_d = nc.dram_tensor("w4", [128, HT, ND], F16, kind="ExternalInput")
    g1_d = nc.dram_tensor("g1", [128, HT], F32, kind="ExternalInput")
    b1_d = nc.dram_tensor("b1", [128, HT], F32, kind="ExternalInput")
    g2_d = nc.dram_tensor("g2", [128, HT], F32, kind="ExternalInput")
    b2_d = nc.dram_tensor("b2", [128, HT], F32, kind="ExternalInput")
    lb_d = nc.dram_tensor("lb", [NCLS], F32, kind="ExternalInput")
    db_d = nc.dram_tensor("db", [ND], F32, kind="ExternalInput")

    logits_d = nc.dram_tensor("logits", [NROI_C, NCLS], F32, kind="ExternalOutput")
    probs_d = nc.dram_tensor("probs", [NROI_C, NCLS], F32, kind="ExternalOutput")
    deltas_d = nc.dram_tensor("deltas", [NROI_C, ND], F32, kind="ExternalOutput")

    w1half_d = [w1a_d, w1b_d]

    with tile.TileContext(nc) as tc:
        with (
            tc.tile_pool(name="persist", bufs=1) as pp,
            tc.tile_pool(name="w1s", bufs=6) as w1p,
            tc.tile_pool(name="scratch", bufs=2) as sp,
            tc.tile_pool(name="psum", bufs=8, space="PSUM") as psp,
            tc.tile_pool(name="dramp", bufs=1, space="DRAM") as dp,
        ):
            # ---- early sync AllReduce: absorbs launch skew + collective entry
            # barrier while GEMM1 runs. It carries EPS/8 per core (sum = EPS
            # exactly: scaling by 8 is exponent-only), and its output is the
            # eps bias of the BN Sqrt — whose real dependency (the stats
            # AllReduce) always completes later, so nothing serializes on it.
            esrc = pp.tile([128, 1], F32, tag="esrc", name="esrc")
            nc.vector.memset(esrc[:], EPS / N_CORES)
            ccin_s = dp.tile([128, 1], F32, tag="ccin_s", name="ccin_s")
            ccout_s = dp.tile([128, 1], F32, tag="ccout_s", name="ccout_s")
            nc.gpsimd.dma_start(ccin_s[:], esrc[:])
            nc.gpsimd.collective_compute(
                "AllReduce",
                mybir.AluOpType.add,
                replica_groups=[list(range(N_CORES))],
                ins=[ccin_s[:].opt()],
                outs=[ccout_s[:].opt()],
            )
            eps_sb = pp.tile([128, 1], F32, tag="epssb", name="epssb")
            nc.gpsimd.dma_start(eps_sb[:], ccout_s[:])

            zero_sb = pp.tile([128, 1], F32, tag="zerosb", name="zerosb")
            nc.vector.memset(zero_sb[:], 0.0)

            # ---- small constants via GpSimd DGE (keeps the Sync queue free
            # for the rois/w1 stream) ----
            g1_sb = pp.tile([128, HT], F32, tag="g1sb", name="g1sb")
            nc.gpsimd.dma_start(g1_sb[:], g1_d[:, :])
            b1_sb = pp.tile([128, HT], F32, tag="b1sb", name="b1sb")
            nc.gpsimd.dma_start(b1_sb[:], b1_d[:, :])
            g2_sb = pp.tile([128, HT], F32, tag="g2sb", name="g2sb")
            nc.gpsimd.dma_start(g2_sb[:], g2_d[:, :])
            b2_sb = pp.tile([128, HT], F32, tag="b2sb", name="b2sb")
            nc.gpsimd.dma_start(b2_sb[:], b2_d[:, :])
            lb_sb = pp.tile([128, NCLS], F32, tag="lbsb", name="lbsb")
            nc.gpsimd.dma_start(lb_sb[:], _bcast_ap(lb_d))
            db_sb = pp.tile([128, ND], F32, tag="dbsb", name="dbsb")
            nc.gpsimd.dma_start(db_sb[:], _bcast_ap(db_d))

            # ---- rois resident; streamed interleaved with w1 half-A chunks.
            # One Sync HWDGE ring sustains ~430GB/s when its queue is kept
            # non-empty, so all large streams go on it in consumption order;
            # the w1 pool's buffer depth provides the issue runway.
            rois_sb = []
            for i in range(NRCH):
                t = pp.tile([128, CH[i], NROI_C], F16, tag=f"rois{i}", name=f"rois{i}")
                rois_sb.append(t)

            w1t = [[None] * NRCH, [None] * NRCH]

            def load_w1_chunk(h, i):
                t = w1p.tile([128, CH[i], 512], F16, tag="w1t", name=f"w1t_{h}_{i}")
                nc.sync.dma_start(t[:], w1half_d[h][:, CH_OFF[i]:CH_OFF[i] + CH[i], :])
                w1t[h][i] = t

            for i in range(NRCH):
                nc.sync.dma_start(
                    rois_sb[i][:], rois_d[:, CH_OFF[i]:CH_OFF[i] + CH[i], :]
                )
                load_w1_chunk(0, i)

            # ---- GEMM1 x1[m(chan), n(roi)] += w1[k, m].T @ roisT[k, n] ----
            x1ps = []
            for m in range(HT):
                x1ps.append(psp.tile([128, NROI_C], F32, tag="ps", name=f"x1ps{m}"))
            x1n = []  # bn1+relu output, fp16
            for m in range(HT):
                x1n.append(pp.tile([128, NROI_C], F16, tag=f"x1n{m}", name=f"x1n{m}"))

            statsg = [None, None]  # AllReduced [sum(4) | sumsq(4)] per half

            def gemm1_half(h):
                for k in range(K1T):
                    ci, co = K2CH[k]
                    wt = w1t[h][ci][:, co, :]
                    rch = rois_sb[ci][:, co, :]
                    for mi in range(4):
                        nc.tensor.matmul(
                            x1ps[4 * h + mi][:],
                            wt[:, mi * 128:(mi + 1) * 128],
                            rch,
                            start=(k == 0),
                            stop=(k == K1T - 1),
                        )
                # local stats: cols 0..3 = sum, 4..7 = sumsq
                stats = pp.tile([128, 8], F32, tag=f"stats{h}", name=f"stats{h}")
                for mi in range(4):
                    nc.vector.reduce_sum(
                        stats[:, mi:mi + 1], x1ps[4 * h + mi][:], axis=AX
                    )
                    sq = sp.tile([128, NROI_C], F32, tag="sqscr", name=f"sq{h}{mi}")
                    nc.scalar.activation(
                        sq[:], x1ps[4 * h + mi][:], AF.Square,
                        bias=zero_sb[:, 0:1],
                        accum_out=stats[:, 4 + mi:5 + mi],
                    )
                ccin = dp.tile([128, 8], F32, tag=f"ccin{h}", name=f"ccin{h}")
                ccout = dp.tile([128, 8], F32, tag=f"ccout{h}", name=f"ccout{h}")
                nc.sync.dma_start(ccin[:], stats[:])
                nc.gpsimd.collective_compute(
                    "AllReduce",
                    mybir.AluOpType.add,
                    replica_groups=[list(range(N_CORES))],
                    ins=[ccin[:].opt()],
                    outs=[ccout[:].opt()],
                )
                sg = pp.tile([128, 8], F32, tag=f"statsg{h}", name=f"statsg{h}")
                nc.sync.dma_start(sg[:], ccout[:])
                statsg[h] = sg

            def bn_affine(sg, nt, gamma, beta, tag):
                """From AllReduced [sum(nt) | sumsq(nt)] compute per-channel
                scale A = gamma*rstd and shift B = beta - mean*A."""
                ms = pp.tile([128, 2 * nt], F32, tag=f"ms{tag}", name=f"ms{tag}")
                nc.vector.tensor_scalar_mul(ms[:], sg[:], 1.0 / N_ROI)
                mean = ms[:, 0:nt]
                e2 = ms[:, nt:2 * nt]
                msq = pp.tile([128, nt], F32, tag=f"msq{tag}", name=f"msq{tag}")
                nc.vector.tensor_mul(msq[:], mean, mean)
                var = pp.tile([128, nt], F32, tag=f"var{tag}", name=f"var{tag}")
                nc.vector.tensor_sub(var[:], e2, msq[:])
                sd = pp.tile([128, nt], F32, tag=f"sd{tag}", name=f"sd{tag}")
                nc.scalar.activation(sd[:], var[:], AF.Sqrt, bias=eps_sb[:, 0:1])
                rstd = pp.tile([128, nt], F32, tag=f"rstd{tag}", name=f"rstd{tag}")
                nc.vector.reciprocal(rstd[:], sd[:])
                A = pp.tile([128, nt], F32, tag=f"A{tag}", name=f"A{tag}")
                nc.vector.tensor_mul(A[:], rstd[:], gamma)
                mA = pp.tile([128, nt], F32, tag=f"mA{tag}", name=f"mA{tag}")
                nc.vector.tensor_mul(mA[:], mean, A[:])
                B = pp.tile([128, nt], F32, tag=f"B{tag}", name=f"B{tag}")
                nc.vector.tensor_sub(B[:], beta, mA[:])
                return A, B

            # half A + its stats/AllReduce
            gemm1_half(0)

            # late preloads for the second phase (emitted after the half-A
            # stream so they don't delay it; all are single large DMAs)
            w2_sb = pp.tile([128, HT, H], F16, tag="w2sb", name="w2sb")
            nc.sync.dma_start(w2_sb[:], w2_d[:, :, :])
            w3_sb = pp.tile([128, HT, NCLS], F16, tag="w3sb", name="w3sb")
            nc.sync.dma_start(w3_sb[:], w3_d[:, :, :])
            w4_sb = pp.tile([128, HT, ND], F16, tag="w4sb", name="w4sb")
            nc.sync.dma_start(w4_sb[:], w4_d[:, :, :])

            # half B
            for i in range(NRCH):
                load_w1_chunk(1, i)
            gemm1_half(1)

            # bn1 apply for half A, then a head start on GEMM2 with k2 in 0..3
            A0, B0 = bn_affine(statsg[0], 4, g1_sb[:, 0:4], b1_sb[:, 0:4], "h0")
            for mi in range(4):
                nc.scalar.activation(
                    x1n[mi][:], x1ps[mi][:], AF.Relu,
                    bias=B0[:, mi:mi + 1], scale=A0[:, mi:mi + 1],
                )

            # ---- GEMM2 x2[m2, n] += w2[k2, m2].T @ x1n[k2, n] ----
            x2ps = []
            for m2 in range(4):
                x2ps.append(psp.tile([128, NROI_C], F32, tag="ps", name=f"x2ps{m2}"))
                for k2 in range(4):
                    nc.tensor.matmul(
                        x2ps[m2][:],
                        w2_sb[:, k2, m2 * 128:(m2 + 1) * 128],
                        x1n[k2][:],
                        start=(k2 == 0),
                        stop=False,
                    )

            # bn1 apply for half B
            A1, B1 = bn_affine(statsg[1], 4, g1_sb[:, 4:8], b1_sb[:, 4:8], "h1")
            for mi in range(4):
                nc.scalar.activation(
                    x1n[4 + mi][:], x1ps[4 + mi][:], AF.Relu,
                    bias=B1[:, mi:mi + 1], scale=A1[:, mi:mi + 1],
                )

            # finish GEMM2 + stats2
            stats2 = pp.tile([128, 16], F32, tag="stats2", name="stats2")

            def stats2_for(m2):
                nc.vector.reduce_sum(stats2[:, m2:m2 + 1], x2ps[m2][:], axis=AX)
                sq = sp.tile([128, NROI_C], F32, tag="sqscr", name=f"sq2_{m2}")
                nc.scalar.activation(
                    sq[:], x2ps[m2][:], AF.Square,
                    bias=zero_sb[:, 0:1],
                    accum_out=stats2[:, 8 + m2:9 + m2],
                )

            for m2 in range(4):
                for k2 in range(4, 8):
                    nc.tensor.matmul(
                        x2ps[m2][:],
                        w2_sb[:, k2, m2 * 128:(m2 + 1) * 128],
                        x1n[k2][:],
                        start=False,
                        stop=(k2 == 7),
                    )
                stats2_for(m2)
            for m2 in range(4, 8):
                x2ps.append(psp.tile([128, NROI_C], F32, tag="ps", name=f"x2ps{m2}"))
                for k2 in range(HT):
                    nc.tensor.matmul(
                        x2ps[m2][:],
                        w2_sb[:, k2, m2 * 128:(m2 + 1) * 128],
                        x1n[k2][:],
                        start=(k2 == 0),
                        stop=(k2 == 7),
                    )
                stats2_for(m2)

            ccin2 = dp.tile([128, 16], F32, tag="ccin2", name="ccin2")
            ccout2 = dp.tile([128, 16], F32, tag="ccout2", name="ccout2")
            nc.sync.dma_start(ccin2[:], stats2[:])
            nc.gpsimd.collective_compute(
                "AllReduce",
                mybir.AluOpType.add,
                replica_groups=[list(range(N_CORES))],
                ins=[ccin2[:].opt()],
                outs=[ccout2[:].opt()],
            )
            statsg2 = pp.tile([128, 16], F32, tag="statsg2", name="statsg2")
            nc.sync.dma_start(statsg2[:], ccout2[:])

            A2, B2 = bn_affine(statsg2, 8, g2_sb[:], b2_sb[:], "l2")
            x2n = []
            for m2 in range(HT):
                t = pp.tile([128, NROI_C], F16, tag=f"x2n{m2}", name=f"x2n{m2}")
                nc.scalar.activation(
                    t[:], x2ps[m2][:], AF.Relu,
                    bias=B2[:, m2:m2 + 1], scale=A2[:, m2:m2 + 1],
                )
                x2n.append(t)

            # ---- heads: out[n, c] += x2n[k, n].T @ w[k, c]; softmax over c ----
            for j, (p0, psz) in enumerate([(0, 128), (128, NROI_C - 128)]):
                pl = psp.tile([128, NCLS], F32, tag="ps", name=f"pl{j}")
                pd = psp.tile([128, ND], F32, tag="ps", name=f"pd{j}")
                for k in range(HT):
                    lhs = x2n[k][:, p0:p0 + psz]
                    nc.tensor.matmul(
                        pl[:psz, :], lhs, w3_sb[:, k, :],
                        start=(k == 0), stop=(k == HT - 1),
                    )
                    nc.tensor.matmul(
                        pd[:psz, :], lhs, w4_sb[:, k, :],
                        start=(k == 0), stop=(k == HT - 1),
                    )
                lg = sp.tile([128, NCLS], F32, tag=f"lg{j}", name=f"lg{j}")
                nc.vector.tensor_add(lg[:psz, :], pl[:psz, :], lb_sb[:psz, :])
                nc.sync.dma_start(logits_d[p0:p0 + psz, :], lg[:psz, :])

                negmx = sp.tile([128, 1], F32, tag=f"nm{j}", name=f"nm{j}")
                nc.vector.reduce_max(negmx[:psz, :], lg[:psz, :], axis=AX, negate=True)
                ex = sp.tile([128, NCLS], F32, tag=f"ex{j}", name=f"ex{j}")
                sume = sp.tile([128, 1], F32, tag=f"se{j}", name=f"se{j}")
                nc.scalar.activation(
                    ex[:psz, :], lg[:psz, :], AF.Exp,
                    bias=negmx[:psz, 0:1], accum_out=sume[:psz, 0:1],
                )
                rcp = sp.tile([128, 1], F32, tag=f"rc{j}", name=f"rc{j}")
                nc.vector.reciprocal(rcp[:psz, :], sume[:psz, :])
                pr = sp.tile([128, NCLS], F32, tag=f"pr{j}", name=f"pr{j}")
                nc.vector.tensor_scalar_mul(pr[:psz, :], ex[:psz, :], rcp[:psz, 0:1])
                nc.sync.dma_start(probs_d[p0:p0 + psz, :], pr[:psz, :])

                dl = sp.tile([128, ND], F32, tag=f"dl{j}", name=f"dl{j}")
                nc.vector.tensor_add(dl[:psz, :], pd[:psz, :], db_sb[:psz, :])
                nc.sync.dma_start(deltas_d[p0:p0 + psz, :], dl[:psz, :])

    nc.compile()
    return nc


_NC_CACHE = None


def _compiled():
    global _NC_CACHE
    if _NC_CACHE is None:
        _NC_CACHE = build()
    return _NC_CACHE


def make_in_maps(inputs):
    """Host-side shard + pack. inputs: dict of full-size float32 arrays."""
    g = {k: np.asarray(v) for k, v in inputs.items()}
    r16 = g["pooled_rois"].reshape(N_ROI, K1).astype(np.float16)
    w1t = g["conv1_w"].reshape(K1, H).astype(np.float16)
    w1t = w1t.reshape(K1T, 128, H).transpose(1, 0, 2)  # [128, 98, 1024]
    w1a = np.ascontiguousarray(w1t[:, :, :512])
    w1b = np.ascontiguousarray(w1t[:, :, 512:])
    w2 = np.ascontiguousarray(
        g["conv2_w"].reshape(H, H).astype(np.float16).reshape(HT, 128, H)
        .transpose(1, 0, 2)
    )
    w3 = np.ascontiguousarray(
        g["logits_w"].astype(np.float16).reshape(HT, 128, NCLS).transpose(1, 0, 2)
    )
    w4 = np.ascontiguousarray(
        g["delta_w"].astype(np.float16).reshape(HT, 128, ND).transpose(1, 0, 2)
    )
    g1 = np.ascontiguousarray(g["bn1_gamma"].astype(np.float32).reshape(HT, 128).T)
    b1 = np.ascontiguousarray(g["bn1_beta"].astype(np.float32).reshape(HT, 128).T)
    g2 = np.ascontiguousarray(g["bn2_gamma"].astype(np.float32).reshape(HT, 128).T)
    b2 = np.ascontiguousarray(g["bn2_beta"].astype(np.float32).reshape(HT, 128).T)
    lb = np.ascontiguousarray(g["logits_b"].astype(np.float32))
    db = np.ascontiguousarray(g["delta_b"].astype(np.float32))
    in_maps = []
    for c in range(N_CORES):
        # [250, 12544] -> roisT [12544, 250] -> [128(p), 98(k), 250(n)]
        shard = r16[c * NROI_C:(c + 1) * NROI_C, :].T
        packed = np.ascontiguousarray(
            shard.reshape(K1T, 128, NROI_C).transpose(1, 0, 2)
        )
        in_maps.append({
            "rois": packed, "w1a": w1a, "w1b": w1b, "w2": w2, "w3": w3, "w4": w4,
            "g1": g1, "b1": b1, "g2": g2, "b2": b2, "lb": lb, "db": db,
        })
    return in_maps


def run_on_cores(inputs, trace=False, tmpdir=None, **kw):
    nc = _compiled()
    in_maps = make_in_maps(inputs)
    res = run_bass_kernel_spmd(
        nc, in_maps, core_ids=list(range(N_CORES)), trace=trace, tmpdir=tmpdir, **kw
    )
    logits = np.concatenate([res.results[c]["logits"] for c in range(N_CORES)], axis=0)
    probs = np.concatenate([res.results[c]["probs"] for c in range(N_CORES)], axis=0)
    deltas = np.concatenate([res.results[c]["deltas"] for c in range(N_CORES)], axis=0)
    out = (
        logits.astype(np.float32),
        probs.astype(np.float32),
        deltas.reshape(N_ROI, NCLS, 4).astype(np.float32),
    )
    return out, res


def kernel(**inputs):
    out, _ = run_on_cores(inputs)
    return out


# revision 18
# speedup vs baseline: 1.1702x; 1.1575x over previous
"""TRN2 Bass/Tile kernel for nn_BBoxHead (2000 ROIs x {GEMM 12544->1024, BN(train),
ReLU, GEMM 1024->1024, BN(train), ReLU, logits/softmax + box-delta heads}).

Strategy: data-parallel over the ROI axis (250 ROIs per core on 8 NeuronCores),
weights replicated, fp16 storage + matmuls with fp32 PSUM accumulation.

Training-mode BN needs batch statistics over all 2000 ROIs, i.e. two cross-core
reductions. On this runtime the in-NEFF collective path is very expensive (its
entry barrier throttles concurrent DMA and each small AllReduce costs 25-45us),
so the computation is split into three NEFF launches with the tiny [1024]-sized
stat reductions done on the host in between:
  p1: GEMM1 -> per-core BN1 partial sums/sumsqs + raw x1 (fp16)
  p2: BN1 apply (host-combined affine) -> GEMM2 -> BN2 partials + raw x2
  p3: BN2 apply -> logits/delta heads -> softmax -> outputs
Each launch is collective-free, so the big GEMM1 stream runs at full DMA rate.

Layout on chip: activations are [channels(partitions), rois(free)], so BN stats
are free-axis reductions and BN apply is a per-partition affine + ReLU in one
scalar-engine activation. The head GEMMs use the activations as the stationary
operand, producing [rois(partitions), classes(free)] so softmax reduces along
the free axis too. All DRAM inputs are host-prepacked so every DMA is
per-partition contiguous with multi-KB descriptors.
"""
import numpy as np

import concourse.bass as bass
import concourse.mybir as mybir
import concourse.tile as tile
from concourse import bacc
from concourse.bass_utils import run_bass_kernel_spmd

N_CORES = 8
N_ROI = 2000
NROI_C = N_ROI // N_CORES  # 250
K1 = 7 * 7 * 256  # 12544
K1T = K1 // 128  # 98
H = 1024
HT = H // 128  # 8
NCLS = 81
ND = NCLS * 4  # 324
EPS = 1e-3
# k-tile chunking of the w1/rois streams: a small first chunk so the PE can
# start early, 14-tile chunks after (28KB/partition contiguous per w1 DMA).
CH = [4] + [14] * 6 + [10]  # sums to 98
CH_OFF = [sum(CH[:i]) for i in range(len(CH))]
NRCH = len(CH)
K2CH = [(i, k - CH_OFF[i]) for i in range(NRCH) for k in range(CH_OFF[i], CH_OFF[i] + CH[i])]
F16 = mybir.dt.float16
F32 = mybir.dt.float32
AX = mybir.AxisListType.X
AF = mybir.ActivationFunctionType


def _bcast_ap(handle, parts=128):
    """DRAM [n] -> broadcast AP [[0, parts], [1, n]] (same row to all partitions)."""
    ap = handle.ap()
    return bass.AP(tensor=ap.tensor, offset=ap.offset, ap=[[0, parts], *ap.ap])


def build_p1():
    """GEMM1 (x1[m,n] += w1[k,m].T @ roisT[k,n]) + BN1 partial stats + raw x1."""
    nc = bacc.Bacc("TRN2", target_bir_lowering=False, debug=False, num_devices=N_CORES)
    rois_d = nc.dram_tensor("rois", [128, K1T, NROI_C], F16, kind="ExternalInput")
    w1_d = nc.dram_tensor("w1", [128, K1T, H], F16, kind="ExternalInput")
    x1_d = nc.dram_tensor("x1", [128, HT, NROI_C], F16, kind="ExternalOutput")
    st_d = nc.dram_tensor("st", [128, 2 * HT], F32, kind="ExternalOutput")

    with tile.TileContext(nc) as tc:
        with (
            tc.tile_pool(name="persist", bufs=1) as pp,
            tc.tile_pool(name="w1s", bufs=4) as w1p,
            tc.tile_pool(name="scratch", bufs=2) as sp,
            tc.tile_pool(name="psum", bufs=8, space="PSUM") as psp,
        ):
            zero_sb = pp.tile([128, 1], F32, tag="zerosb", name="zerosb")
            nc.vector.memset(zero_sb[:], 0.0)

            rois_sb = []
            w1t = []
            for i in range(NRCH):
                r = pp.tile([128, CH[i], NROI_C], F16, tag=f"rois{i}", name=f"rois{i}")
                nc.sync.dma_start(r[:], rois_d[:, CH_OFF[i]:CH_OFF[i] + CH[i], :])
                rois_sb.append(r)
                t = w1p.tile([128, CH[i], H], F16, tag="w1t", name=f"w1t{i}")
                nc.sync.dma_start(t[:], w1_d[:, CH_OFF[i]:CH_OFF[i] + CH[i], :])
                w1t.append(t)

            x1ps = [psp.tile([128, NROI_C], F32, tag="ps", name=f"x1ps{m}")
                    for m in range(HT)]
            for k in range(K1T):
                ci, co = K2CH[k]
                for m in range(HT):
                    nc.tensor.matmul(
                        x1ps[m][:],
                        w1t[ci][:, co, m * 128:(m + 1) * 128],
                        rois_sb[ci][:, co, :],
                        start=(k == 0),
                        stop=(k == K1T - 1),
                    )

            stats = pp.tile([128, 2 * HT], F32, tag="stats", name="stats")
            x1sb = pp.tile([128, HT, NROI_C], F16, tag="x1sb", name="x1sb")
            for m in range(HT):
                nc.vector.reduce_sum(stats[:, m:m + 1], x1ps[m][:], axis=AX)
                sq = sp.tile([128, NROI_C], F32, tag="sqscr", name=f"sq{m}")
                nc.scalar.activation(
                    sq[:], x1ps[m][:], AF.Square,
                    bias=zero_sb[:, 0:1],
                    accum_out=stats[:, HT + m:HT + m + 1],
                )
                nc.scalar.copy(x1sb[:, m, :], x1ps[m][:])
            nc.sync.dma_start(st_d[:, :], stats[:])
            nc.sync.dma_start(x1_d[:, :, :], x1sb[:])
    nc.compile()
    return nc


def build_p2():
    """BN1 apply (affine from host) -> GEMM2 -> BN2 partial stats + raw x2."""
    nc = bacc.Bacc("TRN2", target_bir_lowering=False, debug=False, num_devices=N_CORES)
    x1_d = nc.dram_tensor("x1", [128, HT, NROI_C], F16, kind="ExternalInput")
    w2_d = nc.dram_tensor("w2", [128, HT, H], F16, kind="ExternalInput")
    a1_d = nc.dram_tensor("a1", [128, HT], F32, kind="ExternalInput")
    b1_d = nc.dram_tensor("b1", [128, HT], F32, kind="ExternalInput")
    x2_d = nc.dram_tensor("x2", [128, HT, NROI_C], F16, kind="ExternalOutput")
    st_d = nc.dram_tensor("st", [128, 2 * HT], F32, kind="ExternalOutput")

    with tile.TileContext(nc) as tc:
        with (
            tc.tile_pool(name="persist", bufs=1) as pp,
            tc.tile_pool(name="scratch", bufs=2) as sp,
            tc.tile_pool(name="psum", bufs=8, space="PSUM") as psp,
        ):
            zero_sb = pp.tile([128, 1], F32, tag="zerosb", name="zerosb")
            nc.vector.memset(zero_sb[:], 0.0)
            a1_sb = pp.tile([128, HT], F32, tag="a1sb", name="a1sb")
            nc.gpsimd.dma_start(a1_sb[:], a1_d[:, :])
            b1_sb = pp.tile([128, HT], F32, tag="b1sb", name="b1sb")
            nc.gpsimd.dma_start(b1_sb[:], b1_d[:, :])
            x1_sb = pp.tile([128, HT, NROI_C], F16, tag="x1sb", name="x1sb")
            nc.sync.dma_start(x1_sb[:], x1_d[:, :, :])
            w2_sb = pp.tile([128, HT, H], F16, tag="w2sb", name="w2sb")
            nc.sync.dma_start(w2_sb[:], w2_d[:, :, :])

            x1n = []
            for m in range(HT):
                t = pp.tile([128, NROI_C], F16, tag=f"x1n{m}", name=f"x1n{m}")
                nc.scalar.activation(
                    t[:], x1_sb[:, m, :], AF.Relu,
                    bias=b1_sb[:, m:m + 1], scale=a1_sb[:, m:m + 1],
                )
                x1n.append(t)

            stats = pp.tile([128, 2 * HT], F32, tag="stats", name="stats")
            x2sb = pp.tile([128, HT, NROI_C], F16, tag="x2sb", name="x2sb")
            for m2 in range(HT):
                ps = psp.tile([128, NROI_C], F32, tag="ps", name=f"x2ps{m2}")
                for k2 in range(HT):
                    nc.tensor.matmul(
                        ps[:],
                        w2_sb[:, k2, m2 * 128:(m2 + 1) * 128],
                        x1n[k2][:],
                        start=(k2 == 0),
                        stop=(k2 == HT - 1),
                    )
                nc.vector.reduce_sum(stats[:, m2:m2 + 1], ps[:], axis=AX)
                sq = sp.tile([128, NROI_C], F32, tag="sqscr", name=f"sq{m2}")
                nc.scalar.activation(
                    sq[:], ps[:], AF.Square,
                    bias=zero_sb[:, 0:1],
                    accum_out=stats[:, HT + m2:HT + m2 + 1],
                )
                nc.scalar.copy(x2sb[:, m2, :], ps[:])
            nc.sync.dma_start(st_d[:, :], stats[:])
            nc.sync.dma_start(x2_d[:, :, :], x2sb[:])
    nc.compile()
    return nc


def build_p3():
    """BN2 apply -> logits/delta heads -> softmax -> outputs."""
    nc = bacc.Bacc("TRN2", target_bir_lowering=False, debug=False, num_devices=N_CORES)
    x2_d = nc.dram_tensor("x2", [128, HT, NROI_C], F16, kind="ExternalInput")
    w3_d = nc.dram_tensor("w3", [128, HT, NCLS], F16, kind="ExternalInput")
    w4_d = nc.dram_tensor("w4", [128, HT, ND], F16, kind="ExternalInput")
    a2_d = nc.dram_tensor("a2", [128, HT], F32, kind="ExternalInput")
    b2_d = nc.dram_tensor("b2", [128, HT], F32, kind="ExternalInput")
    lb_d = nc.dram_tensor("lb", [NCLS], F32, kind="ExternalInput")
    db_d = nc.dram_tensor("db", [ND], F32, kind="ExternalInput")
    logits_d = nc.dram_tensor("logits", [NROI_C, NCLS], F32, kind="ExternalOutput")
    probs_d = nc.dram_tensor("probs", [NROI_C, NCLS], F32, kind="ExternalOutput")
    deltas_d = nc.dram_tensor("deltas", [NROI_C, ND], F32, kind="ExternalOutput")

    with tile.TileContext(nc) as tc:
        with (
            tc.tile_pool(name="persist", bufs=1) as pp,
            tc.tile_pool(name="scratch", bufs=2) as sp,
            tc.tile_pool(name="psum", bufs=8, space="PSUM") as psp,
        ):
            a2_sb = pp.tile([128, HT], F32, tag="a2sb", name="a2sb")
            nc.gpsimd.dma_start(a2_sb[:], a2_d[:, :])
            b2_sb = pp.tile([128, HT], F32, tag="b2sb", name="b2sb")
            nc.gpsimd.dma_start(b2_sb[:], b2_d[:, :])
            lb_sb = pp.tile([128, NCLS], F32, tag="lbsb", name="lbsb")
            nc.gpsimd.dma_start(lb_sb[:], _bcast_ap(lb_d))
            db_sb = pp.tile([128, ND], F32, tag="dbsb", name="dbsb")
            nc.gpsimd.dma_start(db_sb[:], _bcast_ap(db_d))
            x2_sb = pp.tile([128, HT, NROI_C], F16, tag="x2sb", name="x2sb")
            nc.sync.dma_start(x2_sb[:], x2_d[:, :, :])
            w3_sb = pp.tile([128, HT, NCLS], F16, tag="w3sb", name="w3sb")
            nc.sync.dma_start(w3_sb[:], w3_d[:, :, :])
            w4_sb = pp.tile([128, HT, ND], F16, tag="w4sb", name="w4sb")
            nc.sync.dma_start(w4_sb[:], w4_d[:, :, :])

            x2n = []
            for m2 in range(HT):
                t = pp.tile([128, NROI_C], F16, tag=f"x2n{m2}", name=f"x2n{m2}")
                nc.scalar.activation(
                    t[:], x2_sb[:, m2, :], AF.Relu,
                    bias=b2_sb[:, m2:m2 + 1], scale=a2_sb[:, m2:m2 + 1],
                )
                x2n.append(t)

            for j, (p0, psz) in enumerate([(0, 128), (128, NROI_C - 128)]):
                pl = psp.tile([128, NCLS], F32, tag="ps", name=f"pl{j}")
                pd = psp.tile([128, ND], F32, tag="ps", name=f"pd{j}")
                for k in range(HT):
                    lhs = x2n[k][:, p0:p0 + psz]
                    nc.tensor.matmul(
                        pl[:psz, :], lhs, w3_sb[:, k, :],
                        start=(k == 0), stop=(k == HT - 1),
                    )
                    nc.tensor.matmul(
                        pd[:psz, :], lhs, w4_sb[:, k, :],
                        start=(k == 0), stop=(k == HT - 1),
                    )
                lg = sp.tile([128, NCLS], F32, tag=f"lg{j}", name=f"lg{j}")
                nc.vector.tensor_add(lg[:psz, :], pl[:psz, :], lb_sb[:psz, :])
                nc.sync.dma_start(logits_d[p0:p0 + psz, :], lg[:psz, :])

                negmx = sp.tile([128, 1], F32, tag=f"nm{j}", name=f"nm{j}")
                nc.vector.reduce_max(negmx[:psz, :], lg[:psz, :], axis=AX, negate=True)
                ex = sp.tile([128, NCLS], F32, tag=f"ex{j}", name=f"ex{j}")
                sume = sp.tile([128, 1], F32, tag=f"se{j}", name=f"se{j}")
                nc.scalar.activation(
                    ex[:psz, :], lg[:psz, :], AF.Exp,
                    bias=negmx[:psz, 0:1], accum_out=sume[:psz, 0:1],
                )
                rcp = sp.tile([128, 1], F32, tag=f"rc{j}", name=f"rc{j}")
                nc.vector.reciprocal(rcp[:psz, :], sume[:psz, :])
                pr = sp.tile([128, NCLS], F32, tag=f"pr{j}", name=f"pr{j}")
                nc.vector.tensor_scalar_mul(pr[:psz, :], ex[:psz, :], rcp[:psz, 0:1])
                nc.sync.dma_start(probs_d[p0:p0 + psz, :], pr[:psz, :])

                dl = sp.tile([128, ND], F32, tag=f"dl{j}", name=f"dl{j}")
                nc.vector.tensor_add(dl[:psz, :], pd[:psz, :], db_sb[:psz, :])
                nc.sync.dma_start(deltas_d[p0:p0 + psz, :], dl[:psz, :])
    nc.compile()
    return nc


_CACHE = {}


def _compiled(which):
    if which not in _CACHE:
        _CACHE[which] = {"p1": build_p1, "p2": build_p2, "p3": build_p3}[which]()
    return _CACHE[which]


def _pack_inputs(inputs):
    g = {k: np.asarray(v) for k, v in inputs.items()}
    r16 = g["pooled_rois"].reshape(N_ROI, K1).astype(np.float16)
    w1 = np.ascontiguousarray(
        g["conv1_w"].reshape(K1, H).astype(np.float16)
        .reshape(K1T, 128, H).transpose(1, 0, 2)
    )
    w2 = np.ascontiguousarray(
        g["conv2_w"].reshape(H, H).astype(np.float16)
        .reshape(HT, 128, H).transpose(1, 0, 2)
    )
    w3 = np.ascontiguousarray(
        g["logits_w"].astype(np.float16).reshape(HT, 128, NCLS).transpose(1, 0, 2)
    )
    w4 = np.ascontiguousarray(
        g["delta_w"].astype(np.float16).reshape(HT, 128, ND).transpose(1, 0, 2)
    )
    rois = []
    for c in range(N_CORES):
        shard = r16[c * NROI_C:(c + 1) * NROI_C, :].T  # [12544, 250]
        rois.append(np.ascontiguousarray(
            shard.reshape(K1T, 128, NROI_C).transpose(1, 0, 2)
        ))
    return g, rois, w1, w2, w3, w4


def _bn_affine_host(stats_list, gamma, beta):
    """Combine per-core [128, 16] partial stats into per-channel affine
    (A = gamma*rstd, B = beta - mean*A), packed as [128, HT] f32."""
    tot = np.sum([s.astype(np.float32) for s in stats_list], axis=0)  # [128, 16]
    sums = tot[:, :HT].T.reshape(H)      # channel = m*128 + p
    sumsq = tot[:, HT:].T.reshape(H)
    mean = (sums / N_ROI).astype(np.float32)
    var = (sumsq / N_ROI).astype(np.float32) - mean * mean
    rstd = 1.0 / np.sqrt(var + np.float32(EPS))
    A = (gamma.astype(np.float32) * rstd).astype(np.float32)
    B = (beta.astype(np.float32) - mean * A).astype(np.float32)
    a_pk = np.ascontiguousarray(A.reshape(HT, 128).T)
    b_pk = np.ascontiguousarray(B.reshape(HT, 128).T)
    return a_pk, b_pk


def run_on_cores(inputs, trace=False, tmpdir=None, **kw):
    g, rois, w1, w2, w3, w4 = _pack_inputs(inputs)
    cores = list(range(N_CORES))

    res1 = run_bass_kernel_spmd(
        _compiled("p1"), [{"rois": rois[c], "w1": w1} for c in cores],
        core_ids=cores, trace=trace,
        tmpdir=(tmpdir + "_p1") if tmpdir else None, **kw,
    )
    a1, b1 = _bn_affine_host(
        [res1.results[c]["st"] for c in cores], g["bn1_gamma"], g["bn1_beta"]
    )
    res2 = run_bass_kernel_spmd(
        _compiled("p2"),
        [{"x1": res1.results[c]["x1"], "w2": w2, "a1": a1, "b1": b1} for c in cores],
        core_ids=cores, trace=trace,
        tmpdir=(tmpdir + "_p2") if tmpdir else None, **kw,
    )
    a2, b2 = _bn_affine_host(
        [res2.results[c]["st"] for c in cores], g["bn2_gamma"], g["bn2_beta"]
    )
    lb = np.ascontiguousarray(g["logits_b"].astype(np.float32))
    db = np.ascontiguousarray(g["delta_b"].astype(np.float32))
    res3 = run_bass_kernel_spmd(
        _compiled("p3"),
        [{"x2": res2.results[c]["x2"], "w3": w3, "w4": w4,
          "a2": a2, "b2": b2, "lb": lb, "db": db} for c in cores],
        core_ids=cores, trace=trace,
        tmpdir=(tmpdir + "_p3") if tmpdir else None, **kw,
    )
    logits = np.concatenate([res3.results[c]["logits"] for c in cores], axis=0)
    probs = np.concatenate([res3.results[c]["probs"] for c in cores], axis=0)
    deltas = np.concatenate([res3.results[c]["deltas"] for c in cores], axis=0)
    out = (
        logits.astype(np.float32),
        probs.astype(np.float32),
        deltas.reshape(N_ROI, NCLS, 4).astype(np.float32),
    )
    return out, (res1, res2, res3)


def kernel(**inputs):
    out, _ = run_on_cores(inputs)
    return out


# revision 23
# speedup vs baseline: 1.2007x; 1.0261x over previous
"""TRN2 Bass/Tile kernel for nn_BBoxHead (2000 ROIs x {GEMM 12544->1024, BN(train),
ReLU, GEMM 1024->1024, BN(train), ReLU, logits/softmax + box-delta heads}).

Strategy: data-parallel over the ROI axis (250 ROIs per core on 8 NeuronCores),
weights replicated, fp16 storage + matmuls with fp32 PSUM accumulation.

Training-mode BN needs batch statistics over all 2000 ROIs, i.e. two cross-core
reductions. On this runtime the in-NEFF collective path is very expensive (its
entry barrier throttles concurrent DMA and each small AllReduce costs 25-45us),
so the computation is split into three NEFF launches with the tiny [1024]-sized
stat reductions done on the host in between:
  p1: GEMM1 -> per-core BN1 partial sums/sumsqs + raw x1 (fp16)
  p2: BN1 apply (host-combined affine) -> GEMM2 -> BN2 partials + raw x2
  p3: BN2 apply -> logits/delta heads -> softmax -> outputs
Each launch is collective-free, so the big GEMM1 stream runs at full DMA rate.

Layout on chip: activations are [channels(partitions), rois(free)], so BN stats
are free-axis reductions and BN apply is a per-partition affine + ReLU in one
scalar-engine activation. The head GEMMs use the activations as the stationary
operand, producing [rois(partitions), classes(free)] so softmax reduces along
the free axis too. All DRAM inputs are host-prepacked so every DMA is
per-partition contiguous with multi-KB descriptors.
"""
import numpy as np

import concourse.bass as bass
import concourse.mybir as mybir
import concourse.tile as tile
from concourse import bacc
from concourse.bass_utils import run_bass_kernel_spmd

N_CORES = 8
N_ROI = 2000
NROI_C = N_ROI // N_CORES  # 250
K1 = 7 * 7 * 256  # 12544
K1T = K1 // 128  # 98
H = 1024
HT = H // 128  # 8
NCLS = 81
ND = NCLS * 4  # 324
EPS = 1e-3
# k-tile chunking of the w1/rois streams: a small first chunk so the PE can
# start early, 16-tile chunks after (32KB/partition contiguous per w1 DMA).
CH = [2] + [16] * 6  # sums to 98
CH_OFF = [sum(CH[:i]) for i in range(len(CH))]
NRCH = len(CH)
K2CH = [(i, k - CH_OFF[i]) for i in range(NRCH) for k in range(CH_OFF[i], CH_OFF[i] + CH[i])]
F16 = mybir.dt.float16
F32 = mybir.dt.float32
AX = mybir.AxisListType.X
AF = mybir.ActivationFunctionType


def _bcast_ap(handle, parts=128):
    """DRAM [n] -> broadcast AP [[0, parts], [1, n]] (same row to all partitions)."""
    ap = handle.ap()
    return bass.AP(tensor=ap.tensor, offset=ap.offset, ap=[[0, parts], *ap.ap])


def build_p1():
    """GEMM1 (x1[m,n] += w1[k,m].T @ roisT[k,n]) + BN1 partial stats + raw x1."""
    nc = bacc.Bacc("TRN2", target_bir_lowering=False, debug=False, num_devices=N_CORES)
    rois_d = nc.dram_tensor("rois", [128, K1T, NROI_C], F16, kind="ExternalInput")
    w1_d = nc.dram_tensor("w1", [128, K1T, H], F16, kind="ExternalInput")
    x1_d = nc.dram_tensor("x1", [128, HT, NROI_C], F16, kind="ExternalOutput")
    st_d = nc.dram_tensor("st", [128, 2 * HT], F32, kind="ExternalOutput")

    with tile.TileContext(nc) as tc:
        with (
            tc.tile_pool(name="persist", bufs=1) as pp,
            tc.tile_pool(name="w1s", bufs=3) as w1p,
            tc.tile_pool(name="scratch", bufs=2) as sp,
            tc.tile_pool(name="psum", bufs=8, space="PSUM") as psp,
        ):
            zero_sb = pp.tile([128, 1], F32, tag="zerosb", name="zerosb")
            nc.vector.memset(zero_sb[:], 0.0)

            rois_sb = []
            w1t = []
            for i in range(NRCH):
                r = pp.tile([128, CH[i], NROI_C], F16, tag=f"rois{i}", name=f"rois{i}")
                nc.sync.dma_start(r[:], rois_d[:, CH_OFF[i]:CH_OFF[i] + CH[i], :])
                rois_sb.append(r)
                t = w1p.tile([128, CH[i], H], F16, tag="w1t", name=f"w1t{i}")
                nc.sync.dma_start(t[:], w1_d[:, CH_OFF[i]:CH_OFF[i] + CH[i], :])
                w1t.append(t)

            x1ps = [psp.tile([128, NROI_C], F32, tag="ps", name=f"x1ps{m}")
                    for m in range(HT)]
            # bulk: k-outer / m-inner (one W1 read); last chunk m-outer so each
            # channel tile's stats/copy/writeout overlaps the remaining matmuls
            for k in range(K1T - CH[-1]):
                ci, co = K2CH[k]
                for m in range(HT):
                    nc.tensor.matmul(
                        x1ps[m][:],
                        w1t[ci][:, co, m * 128:(m + 1) * 128],
                        rois_sb[ci][:, co, :],
                        start=(k == 0),
                        stop=False,
                    )

            stats = pp.tile([128, 2 * HT], F32, tag="stats", name="stats")
            for m in range(HT):
                for k in range(K1T - CH[-1], K1T):
                    ci, co = K2CH[k]
                    nc.tensor.matmul(
                        x1ps[m][:],
                        w1t[ci][:, co, m * 128:(m + 1) * 128],
                        rois_sb[ci][:, co, :],
                        start=False,
                        stop=(k == K1T - 1),
                    )
                nc.vector.reduce_sum(stats[:, m:m + 1], x1ps[m][:], axis=AX)
                sq = sp.tile([128, NROI_C], F32, tag="sqscr", name=f"sq{m}")
                nc.scalar.activation(
                    sq[:], x1ps[m][:], AF.Square,
                    bias=zero_sb[:, 0:1],
                    accum_out=stats[:, HT + m:HT + m + 1],
                )
                x1o = sp.tile([128, NROI_C], F16, tag=f"x1o{m}", name=f"x1o{m}")
                nc.scalar.copy(x1o[:], x1ps[m][:])
                nc.sync.dma_start(x1_d[:, m, :], x1o[:])
            nc.sync.dma_start(st_d[:, :], stats[:])
    nc.compile()
    return nc


def build_p2():
    """BN1 apply (affine from host) -> GEMM2 -> BN2 partial stats + raw x2."""
    nc = bacc.Bacc("TRN2", target_bir_lowering=False, debug=False, num_devices=N_CORES)
    x1_d = nc.dram_tensor("x1", [128, HT, NROI_C], F16, kind="ExternalInput")
    w2_d = nc.dram_tensor("w2", [128, HT, H], F16, kind="ExternalInput")
    a1_d = nc.dram_tensor("a1", [128, HT], F32, kind="ExternalInput")
    b1_d = nc.dram_tensor("b1", [128, HT], F32, kind="ExternalInput")
    x2_d = nc.dram_tensor("x2", [128, HT, NROI_C], F16, kind="ExternalOutput")
    st_d = nc.dram_tensor("st", [128, 2 * HT], F32, kind="ExternalOutput")

    with tile.TileContext(nc) as tc:
        with (
            tc.tile_pool(name="persist", bufs=1) as pp,
            tc.tile_pool(name="scratch", bufs=2) as sp,
            tc.tile_pool(name="psum", bufs=8, space="PSUM") as psp,
        ):
            zero_sb = pp.tile([128, 1], F32, tag="zerosb", name="zerosb")
            nc.vector.memset(zero_sb[:], 0.0)
            a1_sb = pp.tile([128, HT], F32, tag="a1sb", name="a1sb")
            nc.gpsimd.dma_start(a1_sb[:], a1_d[:, :])
            b1_sb = pp.tile([128, HT], F32, tag="b1sb", name="b1sb")
            nc.gpsimd.dma_start(b1_sb[:], b1_d[:, :])

            # k2-outer: per k2, load the x1/w2 slices, apply BN1, then one
            # matmul per output tile — GEMM2 starts after the first slice.
            x2ps = [psp.tile([128, NROI_C], F32, tag="ps", name=f"x2ps{m2}")
                    for m2 in range(HT)]
            x1n = []
            for k2 in range(HT):
                xs = pp.tile([128, NROI_C], F16, tag=f"xs{k2}", name=f"xs{k2}")
                nc.sync.dma_start(xs[:], x1_d[:, k2, :])
                w2k = pp.tile([128, H], F16, tag=f"w2k{k2}", name=f"w2k{k2}")
                nc.sync.dma_start(w2k[:], w2_d[:, k2, :])
                t = pp.tile([128, NROI_C], F16, tag=f"x1n{k2}", name=f"x1n{k2}")
                nc.scalar.activation(
                    t[:], xs[:], AF.Relu,
                    bias=b1_sb[:, k2:k2 + 1], scale=a1_sb[:, k2:k2 + 1],
                )
                x1n.append(t)
                for m2 in range(HT):
                    nc.tensor.matmul(
                        x2ps[m2][:],
                        w2k[:, m2 * 128:(m2 + 1) * 128],
                        t[:],
                        start=(k2 == 0),
                        stop=(k2 == HT - 1),
                    )

            stats = pp.tile([128, 2 * HT], F32, tag="stats", name="stats")
            for m2 in range(HT):
                nc.vector.reduce_sum(stats[:, m2:m2 + 1], x2ps[m2][:], axis=AX)
                sq = sp.tile([128, NROI_C], F32, tag="sqscr", name=f"sq{m2}")
                nc.scalar.activation(
                    sq[:], x2ps[m2][:], AF.Square,
                    bias=zero_sb[:, 0:1],
                    accum_out=stats[:, HT + m2:HT + m2 + 1],
                )
                x2o = sp.tile([128, NROI_C], F16, tag=f"x2o{m2}", name=f"x2o{m2}")
                nc.scalar.copy(x2o[:], x2ps[m2][:])
                nc.sync.dma_start(x2_d[:, m2, :], x2o[:])
            nc.sync.dma_start(st_d[:, :], stats[:])
    nc.compile()
    return nc


def build_p3():
    """BN2 apply -> logits/delta heads -> softmax -> outputs."""
    nc = bacc.Bacc("TRN2", target_bir_lowering=False, debug=False, num_devices=N_CORES)
    x2_d = nc.dram_tensor("x2", [128, HT, NROI_C], F16, kind="ExternalInput")
    w3_d = nc.dram_tensor("w3", [128, HT, NCLS], F16, kind="ExternalInput")
    w4_d = nc.dram_tensor("w4", [128, HT, ND], F16, kind="ExternalInput")
    a2_d = nc.dram_tensor("a2", [128, HT], F32, kind="ExternalInput")
    b2_d = nc.dram_tensor("b2", [128, HT], F32, kind="ExternalInput")
    lb_d = nc.dram_tensor("lb", [NCLS], F32, kind="ExternalInput")
    db_d = nc.dram_tensor("db", [ND], F32, kind="ExternalInput")
    logits_d = nc.dram_tensor("logits", [NROI_C, NCLS], F32, kind="ExternalOutput")
    probs_d = nc.dram_tensor("probs", [NROI_C, NCLS], F32, kind="ExternalOutput")
    deltas_d = nc.dram_tensor("deltas", [NROI_C, ND], F32, kind="ExternalOutput")

    with tile.TileContext(nc) as tc:
        with (
            tc.tile_pool(name="persist", bufs=1) as pp,
            tc.tile_pool(name="scratch", bufs=2) as sp,
            tc.tile_pool(name="psum", bufs=8, space="PSUM") as psp,
        ):
            a2_sb = pp.tile([128, HT], F32, tag="a2sb", name="a2sb")
            nc.gpsimd.dma_start(a2_sb[:], a2_d[:, :])
            b2_sb = pp.tile([128, HT], F32, tag="b2sb", name="b2sb")
            nc.gpsimd.dma_start(b2_sb[:], b2_d[:, :])
            lb_sb = pp.tile([128, NCLS], F32, tag="lbsb", name="lbsb")
            nc.gpsimd.dma_start(lb_sb[:], _bcast_ap(lb_d))
            db_sb = pp.tile([128, ND], F32, tag="dbsb", name="dbsb")
            nc.gpsimd.dma_start(db_sb[:], _bcast_ap(db_d))
            w3_sb = pp.tile([128, HT, NCLS], F16, tag="w3sb", name="w3sb")
            nc.sync.dma_start(w3_sb[:], w3_d[:, :, :])
            w4_sb = pp.tile([128, HT, ND], F16, tag="w4sb", name="w4sb")
            nc.sync.dma_start(w4_sb[:], w4_d[:, :, :])

            x2n = []
            for m2 in range(HT):
                xs = pp.tile([128, NROI_C], F16, tag=f"xs{m2}", name=f"xs{m2}")
                nc.sync.dma_start(xs[:], x2_d[:, m2, :])
                t = pp.tile([128, NROI_C], F16, tag=f"x2n{m2}", name=f"x2n{m2}")
                nc.scalar.activation(
                    t[:], xs[:], AF.Relu,
                    bias=b2_sb[:, m2:m2 + 1], scale=a2_sb[:, m2:m2 + 1],
                )
                x2n.append(t)

            for j, (p0, psz) in enumerate([(0, 128), (128, NROI_C - 128)]):
                pl = psp.tile([128, NCLS], F32, tag="ps", name=f"pl{j}")
                pd = psp.tile([128, ND], F32, tag="ps", name=f"pd{j}")
                for k in range(HT):
                    lhs = x2n[k][:, p0:p0 + psz]
                    nc.tensor.matmul(
                        pl[:psz, :], lhs, w3_sb[:, k, :],
                        start=(k == 0), stop=(k == HT - 1),
                    )
                    nc.tensor.matmul(
                        pd[:psz, :], lhs, w4_sb[:, k, :],
                        start=(k == 0), stop=(k == HT - 1),
                    )
                lg = sp.tile([128, NCLS], F32, tag=f"lg{j}", name=f"lg{j}")
                nc.vector.tensor_add(lg[:psz, :], pl[:psz, :], lb_sb[:psz, :])
                nc.sync.dma_start(logits_d[p0:p0 + psz, :], lg[:psz, :])

                negmx = sp.tile([128, 1], F32, tag=f"nm{j}", name=f"nm{j}")
                nc.vector.reduce_max(negmx[:psz, :], lg[:psz, :], axis=AX, negate=True)
                ex = sp.tile([128, NCLS], F32, tag=f"ex{j}", name=f"ex{j}")
                sume = sp.tile([128, 1], F32, tag=f"se{j}", name=f"se{j}")
                nc.scalar.activation(
                    ex[:psz, :], lg[:psz, :], AF.Exp,
                    bias=negmx[:psz, 0:1], accum_out=sume[:psz, 0:1],
                )
                rcp = sp.tile([128, 1], F32, tag=f"rc{j}", name=f"rc{j}")
                nc.vector.reciprocal(rcp[:psz, :], sume[:psz, :])
                pr = sp.tile([128, NCLS], F32, tag=f"pr{j}", name=f"pr{j}")
                nc.vector.tensor_scalar_mul(pr[:psz, :], ex[:psz, :], rcp[:psz, 0:1])
                nc.sync.dma_start(probs_d[p0:p0 + psz, :], pr[:psz, :])

                dl = sp.tile([128, ND], F32, tag=f"dl{j}", name=f"dl{j}")
                nc.vector.tensor_add(dl[:psz, :], pd[:psz, :], db_sb[:psz, :])
                nc.sync.dma_start(deltas_d[p0:p0 + psz, :], dl[:psz, :])
    nc.compile()
    return nc


_CACHE = {}


def _compiled(which):
    if which not in _CACHE:
        _CACHE[which] = {"p1": build_p1, "p2": build_p2, "p3": build_p3}[which]()
    return _CACHE[which]


def _pack_inputs(inputs):
    g = {k: np.asarray(v) for k, v in inputs.items()}
    r16 = g["pooled_rois"].reshape(N_ROI, K1).astype(np.float16)
    w1 = np.ascontiguousarray(
        g["conv1_w"].reshape(K1, H).astype(np.float16)
        .reshape(K1T, 128, H).transpose(1, 0, 2)
    )
    w2 = np.ascontiguousarray(
        g["conv2_w"].reshape(H, H).astype(np.float16)
        .reshape(HT, 128, H).transpose(1, 0, 2)
    )
    w3 = np.ascontiguousarray(
        g["logits_w"].astype(np.float16).reshape(HT, 128, NCLS).transpose(1, 0, 2)
    )
    w4 = np.ascontiguousarray(
        g["delta_w"].astype(np.float16).reshape(HT, 128, ND).transpose(1, 0, 2)
    )
    rois = []
    for c in range(N_CORES):
        shard = r16[c * NROI_C:(c + 1) * NROI_C, :].T  # [12544, 250]
        rois.append(np.ascontiguousarray(
            shard.reshape(K1T, 128, NROI_C).transpose(1, 0, 2)
        ))
    return g, rois, w1, w2, w3, w4


def _bn_affine_host(stats_list, gamma, beta):
    """Combine per-core [128, 16] partial stats into per-channel affine
    (A = gamma*rstd, B = beta - mean*A), packed as [128, HT] f32."""
    tot = np.sum([s.astype(np.float32) for s in stats_list], axis=0)  # [128, 16]
    sums = tot[:, :HT].T.reshape(H)      # channel = m*128 + p
    sumsq = tot[:, HT:].T.reshape(H)
    mean = (sums / N_ROI).astype(np.float32)
    var = (sumsq / N_ROI).astype(np.float32) - mean * mean
    rstd = 1.0 / np.sqrt(var + np.float32(EPS))
    A = (gamma.astype(np.float32) * rstd).astype(np.float32)
    B = (beta.astype(np.float32) - mean * A).astype(np.float32)
    a_pk = np.ascontiguousarray(A.reshape(HT, 128).T)
    b_pk = np.ascontiguousarray(B.reshape(HT, 128).T)
    return a_pk, b_pk


def run_on_cores(inputs, trace=False, tmpdir=None, **kw):
    g, rois, w1, w2, w3, w4 = _pack_inputs(inputs)
    cores = list(range(N_CORES))

    res1 = run_bass_kernel_spmd(
        _compiled("p1"), [{"rois": rois[c], "w1": w1} for c in cores],
        core_ids=cores, trace=trace,
        tmpdir=(tmpdir + "_p1") if tmpdir else None, **kw,
    )
    a1, b1 = _bn_affine_host(
        [res1.results[c]["st"] for c in cores], g["bn1_gamma"], g["bn1_beta"]
    )
    res2 = run_bass_kernel_spmd(
        _compiled("p2"),
        [{"x1": res1.results[c]["x1"], "w2": w2, "a1": a1, "b1": b1} for c in cores],
        core_ids=cores, trace=trace,
        tmpdir=(tmpdir + "_p2") if tmpdir else None, **kw,
    )
    a2, b2 = _bn_affine_host(
        [res2.results[c]["st"] for c in cores], g["bn2_gamma"], g["bn2_beta"]
    )
    lb = np.ascontiguousarray(g["logits_b"].astype(np.float32))
    db = np.ascontiguousarray(g["delta_b"].astype(np.float32))
    res3 = run_bass_kernel_spmd(
        _compiled("p3"),
        [{"x2": res2.results[c]["x2"], "w3": w3, "w4": w4,
          "a2": a2, "b2": b2, "lb": lb, "db": db} for c in cores],
        core_ids=cores, trace=trace,
        tmpdir=(tmpdir + "_p3") if tmpdir else None, **kw,
    )
    logits = np.concatenate([res3.results[c]["logits"] for c in cores], axis=0)
    probs = np.concatenate([res3.results[c]["probs"] for c in cores], axis=0)
    deltas = np.concatenate([res3.results[c]["deltas"] for c in cores], axis=0)
    out = (
        logits.astype(np.float32),
        probs.astype(np.float32),
        deltas.reshape(N_ROI, NCLS, 4).astype(np.float32),
    )
    return out, (res1, res2, res3)


def kernel(**inputs):
    out, _ = run_on_cores(inputs)
    return out


# revision 24
# speedup vs baseline: 1.2034x; 1.0022x over previous
"""TRN2 Bass/Tile kernel for nn_BBoxHead (2000 ROIs x {GEMM 12544->1024, BN(train),
ReLU, GEMM 1024->1024, BN(train), ReLU, logits/softmax + box-delta heads}).

Strategy: data-parallel over the ROI axis (250 ROIs per core on 8 NeuronCores),
weights replicated, fp16 storage + matmuls with fp32 PSUM accumulation.

Training-mode BN needs batch statistics over all 2000 ROIs, i.e. two cross-core
reductions. On this runtime the in-NEFF collective path is very expensive (its
entry barrier throttles concurrent DMA and each small AllReduce costs 25-45us),
so the computation is split into three NEFF launches with the tiny [1024]-sized
stat reductions done on the host in between:
  p1: GEMM1 -> per-core BN1 partial sums/sumsqs + raw x1 (fp16)
  p2: BN1 apply (host-combined affine) -> GEMM2 -> BN2 partials + raw x2
  p3: BN2 apply -> logits/delta heads -> softmax -> outputs
Each launch is collective-free, so the big GEMM1 stream runs at full DMA rate.

Layout on chip: activations are [channels(partitions), rois(free)], so BN stats
are free-axis reductions and BN apply is a per-partition affine + ReLU in one
scalar-engine activation. The head GEMMs use the activations as the stationary
operand, producing [rois(partitions), classes(free)] so softmax reduces along
the free axis too. All DRAM inputs are host-prepacked so every DMA is
per-partition contiguous with multi-KB descriptors.
"""
import numpy as np

import concourse.bass as bass
import concourse.mybir as mybir
import concourse.tile as tile
from concourse import bacc
from concourse.bass_utils import run_bass_kernel_spmd

N_CORES = 8
N_ROI = 2000
NROI_C = N_ROI // N_CORES  # 250
K1 = 7 * 7 * 256  # 12544
K1T = K1 // 128  # 98
H = 1024
HT = H // 128  # 8
NCLS = 81
ND = NCLS * 4  # 324
EPS = 1e-3
# k-tile chunking of the w1/rois streams: a small first chunk so the PE can
# start early, 16-tile chunks after (32KB/partition contiguous per w1 DMA).
CH = [2, 6, 16, 16, 16, 16, 16, 10]  # sums to 98
CH_OFF = [sum(CH[:i]) for i in range(len(CH))]
NRCH = len(CH)
K2CH = [(i, k - CH_OFF[i]) for i in range(NRCH) for k in range(CH_OFF[i], CH_OFF[i] + CH[i])]
F16 = mybir.dt.float16
F32 = mybir.dt.float32
AX = mybir.AxisListType.X
AF = mybir.ActivationFunctionType


def _bcast_ap(handle, parts=128):
    """DRAM [n] -> broadcast AP [[0, parts], [1, n]] (same row to all partitions)."""
    ap = handle.ap()
    return bass.AP(tensor=ap.tensor, offset=ap.offset, ap=[[0, parts], *ap.ap])


def build_p1():
    """GEMM1 (x1[m,n] += w1[k,m].T @ roisT[k,n]) + BN1 partial stats + raw x1."""
    nc = bacc.Bacc("TRN2", target_bir_lowering=False, debug=False, num_devices=N_CORES)
    rois_d = nc.dram_tensor("rois", [128, K1T, NROI_C], F16, kind="ExternalInput")
    w1_d = nc.dram_tensor("w1", [128, K1T, H], F16, kind="ExternalInput")
    x1_d = nc.dram_tensor("x1", [128, HT, NROI_C], F16, kind="ExternalOutput")
    st_d = nc.dram_tensor("st", [128, 2 * HT], F32, kind="ExternalOutput")

    with tile.TileContext(nc) as tc:
        with (
            tc.tile_pool(name="persist", bufs=1) as pp,
            tc.tile_pool(name="w1s", bufs=3) as w1p,
            tc.tile_pool(name="scratch", bufs=2) as sp,
            tc.tile_pool(name="psum", bufs=8, space="PSUM") as psp,
        ):
            zero_sb = pp.tile([128, 1], F32, tag="zerosb", name="zerosb")
            nc.vector.memset(zero_sb[:], 0.0)

            rois_sb = []
            w1t = []
            for i in range(NRCH):
                r = pp.tile([128, CH[i], NROI_C], F16, tag=f"rois{i}", name=f"rois{i}")
                nc.sync.dma_start(r[:], rois_d[:, CH_OFF[i]:CH_OFF[i] + CH[i], :])
                rois_sb.append(r)
                t = w1p.tile([128, CH[i], H], F16, tag="w1t", name=f"w1t{i}")
                nc.sync.dma_start(t[:], w1_d[:, CH_OFF[i]:CH_OFF[i] + CH[i], :])
                w1t.append(t)

            x1ps = [psp.tile([128, NROI_C], F32, tag="ps", name=f"x1ps{m}")
                    for m in range(HT)]
            # bulk: k-outer / m-inner (one W1 read); last chunk m-outer so each
            # channel tile's stats/copy/writeout overlaps the remaining matmuls
            for k in range(K1T - CH[-1]):
                ci, co = K2CH[k]
                for m in range(HT):
                    nc.tensor.matmul(
                        x1ps[m][:],
                        w1t[ci][:, co, m * 128:(m + 1) * 128],
                        rois_sb[ci][:, co, :],
                        start=(k == 0),
                        stop=False,
                    )

            stats = pp.tile([128, 2 * HT], F32, tag="stats", name="stats")
            for m in range(HT):
                for k in range(K1T - CH[-1], K1T):
                    ci, co = K2CH[k]
                    nc.tensor.matmul(
                        x1ps[m][:],
                        w1t[ci][:, co, m * 128:(m + 1) * 128],
                        rois_sb[ci][:, co, :],
                        start=False,
                        stop=(k == K1T - 1),
                    )
                nc.vector.reduce_sum(stats[:, m:m + 1], x1ps[m][:], axis=AX)
                sq = sp.tile([128, NROI_C], F32, tag="sqscr", name=f"sq{m}")
                nc.scalar.activation(
                    sq[:], x1ps[m][:], AF.Square,
                    bias=zero_sb[:, 0:1],
                    accum_out=stats[:, HT + m:HT + m + 1],
                )
                x1o = sp.tile([128, NROI_C], F16, tag=f"x1o{m}", name=f"x1o{m}")
                nc.scalar.copy(x1o[:], x1ps[m][:])
                nc.sync.dma_start(x1_d[:, m, :], x1o[:])
            nc.sync.dma_start(st_d[:, :], stats[:])
    nc.compile()
    return nc


def build_p2():
    """BN1 apply (affine from host) -> GEMM2 -> BN2 partial stats + raw x2."""
    nc = bacc.Bacc("TRN2", target_bir_lowering=False, debug=False, num_devices=N_CORES)
    x1_d = nc.dram_tensor("x1", [128, HT, NROI_C], F16, kind="ExternalInput")
    w2_d = nc.dram_tensor("w2", [128, HT, H], F16, kind="ExternalInput")
    a1_d = nc.dram_tensor("a1", [128, HT], F32, kind="ExternalInput")
    b1_d = nc.dram_tensor("b1", [128, HT], F32, kind="ExternalInput")
    x2_d = nc.dram_tensor("x2", [128, HT, NROI_C], F16, kind="ExternalOutput")
    st_d = nc.dram_tensor("st", [128, 2 * HT], F32, kind="ExternalOutput")

    with tile.TileContext(nc) as tc:
        with (
            tc.tile_pool(name="persist", bufs=1) as pp,
            tc.tile_pool(name="scratch", bufs=2) as sp,
            tc.tile_pool(name="psum", bufs=8, space="PSUM") as psp,
        ):
            zero_sb = pp.tile([128, 1], F32, tag="zerosb", name="zerosb")
            nc.vector.memset(zero_sb[:], 0.0)
            a1_sb = pp.tile([128, HT], F32, tag="a1sb", name="a1sb")
            nc.gpsimd.dma_start(a1_sb[:], a1_d[:, :])
            b1_sb = pp.tile([128, HT], F32, tag="b1sb", name="b1sb")
            nc.gpsimd.dma_start(b1_sb[:], b1_d[:, :])

            # k2-outer: per k2, load the x1/w2 slices, apply BN1, then one
            # matmul per output tile — GEMM2 starts after the first slice.
            x2ps = [psp.tile([128, NROI_C], F32, tag="ps", name=f"x2ps{m2}")
                    for m2 in range(HT)]
            x1n = []
            for k2 in range(HT):
                xs = pp.tile([128, NROI_C], F16, tag=f"xs{k2}", name=f"xs{k2}")
                nc.sync.dma_start(xs[:], x1_d[:, k2, :])
                w2k = pp.tile([128, H], F16, tag=f"w2k{k2}", name=f"w2k{k2}")
                nc.sync.dma_start(w2k[:], w2_d[:, k2, :])
                t = pp.tile([128, NROI_C], F16, tag=f"x1n{k2}", name=f"x1n{k2}")
                nc.scalar.activation(
                    t[:], xs[:], AF.Relu,
                    bias=b1_sb[:, k2:k2 + 1], scale=a1_sb[:, k2:k2 + 1],
                )
                x1n.append(t)
                for m2 in range(HT):
                    nc.tensor.matmul(
                        x2ps[m2][:],
                        w2k[:, m2 * 128:(m2 + 1) * 128],
                        t[:],
                        start=(k2 == 0),
                        stop=(k2 == HT - 1),
                    )

            stats = pp.tile([128, 2 * HT], F32, tag="stats", name="stats")
            for m2 in range(HT):
                nc.vector.reduce_sum(stats[:, m2:m2 + 1], x2ps[m2][:], axis=AX)
                sq = sp.tile([128, NROI_C], F32, tag="sqscr", name=f"sq{m2}")
                nc.scalar.activation(
                    sq[:], x2ps[m2][:], AF.Square,
                    bias=zero_sb[:, 0:1],
                    accum_out=stats[:, HT + m2:HT + m2 + 1],
                )
                x2o = sp.tile([128, NROI_C], F16, tag=f"x2o{m2}", name=f"x2o{m2}")
                nc.scalar.copy(x2o[:], x2ps[m2][:])
                nc.sync.dma_start(x2_d[:, m2, :], x2o[:])
            nc.sync.dma_start(st_d[:, :], stats[:])
    nc.compile()
    return nc


def build_p3():
    """BN2 apply -> logits/delta heads -> softmax -> outputs."""
    nc = bacc.Bacc("TRN2", target_bir_lowering=False, debug=False, num_devices=N_CORES)
    x2_d = nc.dram_tensor("x2", [128, HT, NROI_C], F16, kind="ExternalInput")
    w3_d = nc.dram_tensor("w3", [128, HT, NCLS], F16, kind="ExternalInput")
    w4_d = nc.dram_tensor("w4", [128, HT, ND], F16, kind="ExternalInput")
    a2_d = nc.dram_tensor("a2", [128, HT], F32, kind="ExternalInput")
    b2_d = nc.dram_tensor("b2", [128, HT], F32, kind="ExternalInput")
    lb_d = nc.dram_tensor("lb", [NCLS], F32, kind="ExternalInput")
    db_d = nc.dram_tensor("db", [ND], F32, kind="ExternalInput")
    logits_d = nc.dram_tensor("logits", [NROI_C, NCLS], F32, kind="ExternalOutput")
    probs_d = nc.dram_tensor("probs", [NROI_C, NCLS], F32, kind="ExternalOutput")
    deltas_d = nc.dram_tensor("deltas", [NROI_C, ND], F32, kind="ExternalOutput")

    with tile.TileContext(nc) as tc:
        with (
            tc.tile_pool(name="persist", bufs=1) as pp,
            tc.tile_pool(name="scratch", bufs=2) as sp,
            tc.tile_pool(name="psum", bufs=8, space="PSUM") as psp,
        ):
            a2_sb = pp.tile([128, HT], F32, tag="a2sb", name="a2sb")
            nc.gpsimd.dma_start(a2_sb[:], a2_d[:, :])
            b2_sb = pp.tile([128, HT], F32, tag="b2sb", name="b2sb")
            nc.gpsimd.dma_start(b2_sb[:], b2_d[:, :])
            lb_sb = pp.tile([128, NCLS], F32, tag="lbsb", name="lbsb")
            nc.gpsimd.dma_start(lb_sb[:], _bcast_ap(lb_d))
            db_sb = pp.tile([128, ND], F32, tag="dbsb", name="dbsb")
            nc.gpsimd.dma_start(db_sb[:], _bcast_ap(db_d))
            w3_sb = pp.tile([128, HT, NCLS], F16, tag="w3sb", name="w3sb")
            nc.sync.dma_start(w3_sb[:], w3_d[:, :, :])
            w4_sb = pp.tile([128, HT, ND], F16, tag="w4sb", name="w4sb")
            nc.sync.dma_start(w4_sb[:], w4_d[:, :, :])

            x2n = []
            for m2 in range(HT):
                xs = pp.tile([128, NROI_C], F16, tag=f"xs{m2}", name=f"xs{m2}")
                nc.sync.dma_start(xs[:], x2_d[:, m2, :])
                t = pp.tile([128, NROI_C], F16, tag=f"x2n{m2}", name=f"x2n{m2}")
                nc.scalar.activation(
                    t[:], xs[:], AF.Relu,
                    bias=b2_sb[:, m2:m2 + 1], scale=a2_sb[:, m2:m2 + 1],
                )
                x2n.append(t)

            for j, (p0, psz) in enumerate([(0, 128), (128, NROI_C - 128)]):
                pl = psp.tile([128, NCLS], F32, tag="ps", name=f"pl{j}")
                pd = psp.tile([128, ND], F32, tag="ps", name=f"pd{j}")
                for k in range(HT):
                    lhs = x2n[k][:, p0:p0 + psz]
                    nc.tensor.matmul(
                        pl[:psz, :], lhs, w3_sb[:, k, :],
                        start=(k == 0), stop=(k == HT - 1),
                    )
                    nc.tensor.matmul(
                        pd[:psz, :], lhs, w4_sb[:, k, :],
                        start=(k == 0), stop=(k == HT - 1),
                    )
                lg = sp.tile([128, NCLS], F32, tag=f"lg{j}", name=f"lg{j}")
                nc.vector.tensor_add(lg[:psz, :], pl[:psz, :], lb_sb[:psz, :])
                nc.sync.dma_start(logits_d[p0:p0 + psz, :], lg[:psz, :])

                negmx = sp.tile([128, 1], F32, tag=f"nm{j}", name=f"nm{j}")
                nc.vector.reduce_max(negmx[:psz, :], lg[:psz, :], axis=AX, negate=True)
                ex = sp.tile([128, NCLS], F32, tag=f"ex{j}", name=f"ex{j}")
                sume = sp.tile([128, 1], F32, tag=f"se{j}", name=f"se{j}")
                nc.scalar.activation(
                    ex[:psz, :], lg[:psz, :], AF.Exp,
                    bias=negmx[:psz, 0:1], accum_out=sume[:psz, 0:1],
                )
                rcp = sp.tile([128, 1], F32, tag=f"rc{j}", name=f"rc{j}")
                nc.vector.reciprocal(rcp[:psz, :], sume[:psz, :])
                pr = sp.tile([128, NCLS], F32, tag=f"pr{j}", name=f"pr{j}")
                nc.vector.tensor_scalar_mul(pr[:psz, :], ex[:psz, :], rcp[:psz, 0:1])
                nc.sync.dma_start(probs_d[p0:p0 + psz, :], pr[:psz, :])

                dl = sp.tile([128, ND], F32, tag=f"dl{j}", name=f"dl{j}")
                nc.vector.tensor_add(dl[:psz, :], pd[:psz, :], db_sb[:psz, :])
                nc.sync.dma_start(deltas_d[p0:p0 + psz, :], dl[:psz, :])
    nc.compile()
    return nc


_CACHE = {}


def _compiled(which):
    if which not in _CACHE:
        _CACHE[which] = {"p1": build_p1, "p2": build_p2, "p3": build_p3}[which]()
    return _CACHE[which]


def _pack_inputs(inputs):
    g = {k: np.asarray(v) for k, v in inputs.items()}
    r16 = g["pooled_rois"].reshape(N_ROI, K1).astype(np.float16)
    w1 = np.ascontiguousarray(
        g["conv1_w"].reshape(K1, H).astype(np.float16)
        .reshape(K1T, 128, H).transpose(1, 0, 2)
    )
    w2 = np.ascontiguousarray(
        g["conv2_w"].reshape(H, H).astype(np.float16)
        .reshape(HT, 128, H).transpose(1, 0, 2)
    )
    w3 = np.ascontiguousarray(
        g["logits_w"].astype(np.float16).reshape(HT, 128, NCLS).transpose(1, 0, 2)
    )
    w4 = np.ascontiguousarray(
        g["delta_w"].astype(np.float16).reshape(HT, 128, ND).transpose(1, 0, 2)
    )
    rois = []
    for c in range(N_CORES):
        shard = r16[c * NROI_C:(c + 1) * NROI_C, :].T  # [12544, 250]
        rois.append(np.ascontiguousarray(
            shard.reshape(K1T, 128, NROI_C).transpose(1, 0, 2)
        ))
    return g, rois, w1, w2, w3, w4


def _bn_affine_host(stats_list, gamma, beta):
    """Combine per-core [128, 16] partial stats into per-channel affine
    (A = gamma*rstd, B = beta - mean*A), packed as [128, HT] f32."""
    tot = np.sum([s.astype(np.float32) for s in stats_list], axis=0)  # [128, 16]
    sums = tot[:, :HT].T.reshape(H)      # channel = m*128 + p
    sumsq = tot[:, HT:].T.reshape(H)
    mean = (sums / N_ROI).astype(np.float32)
    var = (sumsq / N_ROI).astype(np.float32) - mean * mean
    rstd = 1.0 / np.sqrt(var + np.float32(EPS))
    A = (gamma.astype(np.float32) * rstd).astype(np.float32)
    B = (beta.astype(np.float32) - mean * A).astype(np.float32)
    a_pk = np.ascontiguousarray(A.reshape(HT, 128).T)
    b_pk = np.ascontiguousarray(B.reshape(HT, 128).T)
    return a_pk, b_pk


def run_on_cores(inputs, trace=False, tmpdir=None, **kw):
    g, rois, w1, w2, w3, w4 = _pack_inputs(inputs)
    cores = list(range(N_CORES))

    res1 = run_bass_kernel_spmd(
        _compiled("p1"), [{"rois": rois[c], "w1": w1} for c in cores],
        core_ids=cores, trace=trace,
        tmpdir=(tmpdir + "_p1") if tmpdir else None, **kw,
    )
    a1, b1 = _bn_affine_host(
        [res1.results[c]["st"] for c in cores], g["bn1_gamma"], g["bn1_beta"]
    )
    res2 = run_bass_kernel_spmd(
        _compiled("p2"),
        [{"x1": res1.results[c]["x1"], "w2": w2, "a1": a1, "b1": b1} for c in cores],
        core_ids=cores, trace=trace,
        tmpdir=(tmpdir + "_p2") if tmpdir else None, **kw,
    )
    a2, b2 = _bn_affine_host(
        [res2.results[c]["st"] for c in cores], g["bn2_gamma"], g["bn2_beta"]
    )
    lb = np.ascontiguousarray(g["logits_b"].astype(np.float32))
    db = np.ascontiguousarray(g["delta_b"].astype(np.float32))
    res3 = run_bass_kernel_spmd(
        _compiled("p3"),
        [{"x2": res2.results[c]["x2"], "w3": w3, "w4": w4,
          "a2": a2, "b2": b2, "lb": lb, "db": db} for c in cores],
        core_ids=cores, trace=trace,
        tmpdir=(tmpdir + "_p3") if tmpdir else None, **kw,
    )
    logits = np.concatenate([res3.results[c]["logits"] for c in cores], axis=0)
    probs = np.concatenate([res3.results[c]["probs"] for c in cores], axis=0)
    deltas = np.concatenate([res3.results[c]["deltas"] for c in cores], axis=0)
    out = (
        logits.astype(np.float32),
        probs.astype(np.float32),
        deltas.reshape(N_ROI, NCLS, 4).astype(np.float32),
    )
    return out, (res1, res2, res3)


def kernel(**inputs):
    out, _ = run_on_cores(inputs)
    return out


# revision 25
# speedup vs baseline: 1.2908x; 1.0727x over previous
"""TRN2 Bass/Tile kernel for nn_BBoxHead (2000 ROIs x {GEMM 12544->1024, BN(train),
ReLU, GEMM 1024->1024, BN(train), ReLU, logits/softmax + box-delta heads}).

Strategy: data-parallel over the ROI axis (250 ROIs per core on 8 NeuronCores),
weights replicated, fp16 storage + matmuls with fp32 PSUM accumulation.

Training-mode BN needs batch statistics over all 2000 ROIs, i.e. two cross-core
reductions. On this runtime the in-NEFF collective path is very expensive (its
entry barrier throttles concurrent DMA and each small AllReduce costs 25-45us),
so the computation is split into three NEFF launches with the tiny [1024]-sized
stat reductions done on the host in between:
  p1: GEMM1 -> per-core BN1 partial sums/sumsqs + raw x1 (fp16)
  p2: BN1 apply (host-combined affine) -> GEMM2 -> BN2 partials + raw x2
  p3: BN2 apply -> logits/delta heads -> softmax -> outputs
Each launch is collective-free, so the big GEMM1 stream runs at full DMA rate.

Layout on chip: activations are [channels(partitions), rois(free)], so BN stats
are free-axis reductions and BN apply is a per-partition affine + ReLU in one
scalar-engine activation. The head GEMMs use the activations as the stationary
operand, producing [rois(partitions), classes(free)] so softmax reduces along
the free axis too. All DRAM inputs are host-prepacked so every DMA is
per-partition contiguous with multi-KB descriptors.
"""
import numpy as np

import concourse.bass as bass
import concourse.mybir as mybir
import concourse.tile as tile
from concourse import bacc
from concourse.bass_utils import run_bass_kernel_spmd

N_CORES = 8
N_ROI = 2000
NROI_C = N_ROI // N_CORES  # 250
K1 = 7 * 7 * 256  # 12544
K1T = K1 // 128  # 98
H = 1024
HT = H // 128  # 8
NCLS = 81
ND = NCLS * 4  # 324
EPS = 1e-3
# k-tile chunking of the w1/rois streams: a small first chunk so the PE can
# start early, 16-tile chunks after (32KB/partition contiguous per w1 DMA).
CH = [4, 4, 5, 5, 6, 6, 7, 8, 8, 9, 10, 11, 12, 3]  # sums to 98
CH_OFF = [sum(CH[:i]) for i in range(len(CH))]
NRCH = len(CH)
K2CH = [(i, k - CH_OFF[i]) for i in range(NRCH) for k in range(CH_OFF[i], CH_OFF[i] + CH[i])]
F16 = mybir.dt.float16
F32 = mybir.dt.float32
AX = mybir.AxisListType.X
AF = mybir.ActivationFunctionType


def _bcast_ap(handle, parts=128):
    """DRAM [n] -> broadcast AP [[0, parts], [1, n]] (same row to all partitions)."""
    ap = handle.ap()
    return bass.AP(tensor=ap.tensor, offset=ap.offset, ap=[[0, parts], *ap.ap])


def build_p1():
    """GEMM1 (x1[m,n] += w1[k,m].T @ roisT[k,n]) + BN1 partial stats + raw x1."""
    nc = bacc.Bacc("TRN2", target_bir_lowering=False, debug=False, num_devices=N_CORES)
    rois_d = nc.dram_tensor("rois", [128, K1T, NROI_C], F16, kind="ExternalInput")
    w1_d = nc.dram_tensor("w1", [128, K1T, H], F16, kind="ExternalInput")
    x1_d = nc.dram_tensor("x1", [128, HT, NROI_C], F16, kind="ExternalOutput")
    st_d = nc.dram_tensor("st", [128, 2 * HT], F32, kind="ExternalOutput")

    with tile.TileContext(nc) as tc:
        with (
            tc.tile_pool(name="persist", bufs=1) as pp,
            tc.tile_pool(name="w1s", bufs=3) as w1p,
            tc.tile_pool(name="scratch", bufs=2) as sp,
            tc.tile_pool(name="psum", bufs=8, space="PSUM") as psp,
        ):
            zero_sb = pp.tile([128, 1], F32, tag="zerosb", name="zerosb")
            nc.vector.memset(zero_sb[:], 0.0)

            rois_sb = []
            w1t = []
            for i in range(NRCH):
                r = pp.tile([128, CH[i], NROI_C], F16, tag=f"rois{i}", name=f"rois{i}")
                nc.sync.dma_start(r[:], rois_d[:, CH_OFF[i]:CH_OFF[i] + CH[i], :])
                rois_sb.append(r)
                t = w1p.tile([128, CH[i], H], F16, tag="w1t", name=f"w1t{i}")
                nc.sync.dma_start(t[:], w1_d[:, CH_OFF[i]:CH_OFF[i] + CH[i], :])
                w1t.append(t)

            x1ps = [psp.tile([128, NROI_C], F32, tag="ps", name=f"x1ps{m}")
                    for m in range(HT)]
            # bulk: k-outer / m-inner (one W1 read); last chunk m-outer so each
            # channel tile's stats/copy/writeout overlaps the remaining matmuls
            for k in range(K1T - CH[-1]):
                ci, co = K2CH[k]
                for m in range(HT):
                    nc.tensor.matmul(
                        x1ps[m][:],
                        w1t[ci][:, co, m * 128:(m + 1) * 128],
                        rois_sb[ci][:, co, :],
                        start=(k == 0),
                        stop=False,
                    )

            stats = pp.tile([128, 2 * HT], F32, tag="stats", name="stats")
            for m in range(HT):
                for k in range(K1T - CH[-1], K1T):
                    ci, co = K2CH[k]
                    nc.tensor.matmul(
                        x1ps[m][:],
                        w1t[ci][:, co, m * 128:(m + 1) * 128],
                        rois_sb[ci][:, co, :],
                        start=False,
                        stop=(k == K1T - 1),
                    )
                nc.vector.reduce_sum(stats[:, m:m + 1], x1ps[m][:], axis=AX)
                sq = sp.tile([128, NROI_C], F32, tag="sqscr", name=f"sq{m}")
                nc.scalar.activation(
                    sq[:], x1ps[m][:], AF.Square,
                    bias=zero_sb[:, 0:1],
                    accum_out=stats[:, HT + m:HT + m + 1],
                )
                x1o = sp.tile([128, NROI_C], F16, tag=f"x1o{m}", name=f"x1o{m}")
                nc.scalar.copy(x1o[:], x1ps[m][:])
                nc.sync.dma_start(x1_d[:, m, :], x1o[:])
            nc.sync.dma_start(st_d[:, :], stats[:])
    nc.compile()
    return nc


def build_p2():
    """BN1 apply (affine from host) -> GEMM2 -> BN2 partial stats + raw x2."""
    nc = bacc.Bacc("TRN2", target_bir_lowering=False, debug=False, num_devices=N_CORES)
    x1_d = nc.dram_tensor("x1", [128, HT, NROI_C], F16, kind="ExternalInput")
    w2_d = nc.dram_tensor("w2", [128, HT, H], F16, kind="ExternalInput")
    a1_d = nc.dram_tensor("a1", [128, HT], F32, kind="ExternalInput")
    b1_d = nc.dram_tensor("b1", [128, HT], F32, kind="ExternalInput")
    x2_d = nc.dram_tensor("x2", [128, HT, NROI_C], F16, kind="ExternalOutput")
    st_d = nc.dram_tensor("st", [128, 2 * HT], F32, kind="ExternalOutput")

    with tile.TileContext(nc) as tc:
        with (
            tc.tile_pool(name="persist", bufs=1) as pp,
            tc.tile_pool(name="scratch", bufs=2) as sp,
            tc.tile_pool(name="psum", bufs=8, space="PSUM") as psp,
        ):
            zero_sb = pp.tile([128, 1], F32, tag="zerosb", name="zerosb")
            nc.vector.memset(zero_sb[:], 0.0)
            a1_sb = pp.tile([128, HT], F32, tag="a1sb", name="a1sb")
            nc.gpsimd.dma_start(a1_sb[:], a1_d[:, :])
            b1_sb = pp.tile([128, HT], F32, tag="b1sb", name="b1sb")
            nc.gpsimd.dma_start(b1_sb[:], b1_d[:, :])

            # k2-outer: per k2, load the x1/w2 slices, apply BN1, then one
            # matmul per output tile — GEMM2 starts after the first slice.
            x2ps = [psp.tile([128, NROI_C], F32, tag="ps", name=f"x2ps{m2}")
                    for m2 in range(HT)]
            x1n = []
            for k2 in range(HT):
                xs = pp.tile([128, NROI_C], F16, tag=f"xs{k2}", name=f"xs{k2}")
                nc.sync.dma_start(xs[:], x1_d[:, k2, :])
                w2k = pp.tile([128, H], F16, tag=f"w2k{k2}", name=f"w2k{k2}")
                nc.sync.dma_start(w2k[:], w2_d[:, k2, :])
                t = pp.tile([128, NROI_C], F16, tag=f"x1n{k2}", name=f"x1n{k2}")
                nc.scalar.activation(
                    t[:], xs[:], AF.Relu,
                    bias=b1_sb[:, k2:k2 + 1], scale=a1_sb[:, k2:k2 + 1],
                )
                x1n.append(t)
                for m2 in range(HT):
                    nc.tensor.matmul(
                        x2ps[m2][:],
                        w2k[:, m2 * 128:(m2 + 1) * 128],
                        t[:],
                        start=(k2 == 0),
                        stop=(k2 == HT - 1),
                    )

            stats = pp.tile([128, 2 * HT], F32, tag="stats", name="stats")
            for m2 in range(HT):
                nc.vector.reduce_sum(stats[:, m2:m2 + 1], x2ps[m2][:], axis=AX)
                sq = sp.tile([128, NROI_C], F32, tag="sqscr", name=f"sq{m2}")
                nc.scalar.activation(
                    sq[:], x2ps[m2][:], AF.Square,
                    bias=zero_sb[:, 0:1],
                    accum_out=stats[:, HT + m2:HT + m2 + 1],
                )
                x2o = sp.tile([128, NROI_C], F16, tag=f"x2o{m2}", name=f"x2o{m2}")
                nc.scalar.copy(x2o[:], x2ps[m2][:])
                nc.sync.dma_start(x2_d[:, m2, :], x2o[:])
            nc.sync.dma_start(st_d[:, :], stats[:])
    nc.compile()
    return nc


def build_p3():
    """BN2 apply -> logits/delta heads -> softmax -> outputs."""
    nc = bacc.Bacc("TRN2", target_bir_lowering=False, debug=False, num_devices=N_CORES)
    x2_d = nc.dram_tensor("x2", [128, HT, NROI_C], F16, kind="ExternalInput")
    w3_d = nc.dram_tensor("w3", [128, HT, NCLS], F16, kind="ExternalInput")
    w4_d = nc.dram_tensor("w4", [128, HT, ND], F16, kind="ExternalInput")
    a2_d = nc.dram_tensor("a2", [128, HT], F32, kind="ExternalInput")
    b2_d = nc.dram_tensor("b2", [128, HT], F32, kind="ExternalInput")
    lb_d = nc.dram_tensor("lb", [NCLS], F32, kind="ExternalInput")
    db_d = nc.dram_tensor("db", [ND], F32, kind="ExternalInput")
    logits_d = nc.dram_tensor("logits", [NROI_C, NCLS], F32, kind="ExternalOutput")
    probs_d = nc.dram_tensor("probs", [NROI_C, NCLS], F32, kind="ExternalOutput")
    deltas_d = nc.dram_tensor("deltas", [NROI_C, ND], F32, kind="ExternalOutput")

    with tile.TileContext(nc) as tc:
        with (
            tc.tile_pool(name="persist", bufs=1) as pp,
            tc.tile_pool(name="scratch", bufs=2) as sp,
            tc.tile_pool(name="psum", bufs=8, space="PSUM") as psp,
        ):
            a2_sb = pp.tile([128, HT], F32, tag="a2sb", name="a2sb")
            nc.gpsimd.dma_start(a2_sb[:], a2_d[:, :])
            b2_sb = pp.tile([128, HT], F32, tag="b2sb", name="b2sb")
            nc.gpsimd.dma_start(b2_sb[:], b2_d[:, :])
            lb_sb = pp.tile([128, NCLS], F32, tag="lbsb", name="lbsb")
            nc.gpsimd.dma_start(lb_sb[:], _bcast_ap(lb_d))
            db_sb = pp.tile([128, ND], F32, tag="dbsb", name="dbsb")
            nc.gpsimd.dma_start(db_sb[:], _bcast_ap(db_d))
            w3_sb = pp.tile([128, HT, NCLS], F16, tag="w3sb", name="w3sb")
            nc.sync.dma_start(w3_sb[:], w3_d[:, :, :])
            w4_sb = pp.tile([128, HT, ND], F16, tag="w4sb", name="w4sb")
            nc.sync.dma_start(w4_sb[:], w4_d[:, :, :])

            x2n = []
            for m2 in range(HT):
                xs = pp.tile([128, NROI_C], F16, tag=f"xs{m2}", name=f"xs{m2}")
                nc.sync.dma_start(xs[:], x2_d[:, m2, :])
                t = pp.tile([128, NROI_C], F16, tag=f"x2n{m2}", name=f"x2n{m2}")
                nc.scalar.activation(
                    t[:], xs[:], AF.Relu,
                    bias=b2_sb[:, m2:m2 + 1], scale=a2_sb[:, m2:m2 + 1],
                )
                x2n.append(t)

            for j, (p0, psz) in enumerate([(0, 128), (128, NROI_C - 128)]):
                pl = psp.tile([128, NCLS], F32, tag="ps", name=f"pl{j}")
                pd = psp.tile([128, ND], F32, tag="ps", name=f"pd{j}")
                for k in range(HT):
                    lhs = x2n[k][:, p0:p0 + psz]
                    nc.tensor.matmul(
                        pl[:psz, :], lhs, w3_sb[:, k, :],
                        start=(k == 0), stop=(k == HT - 1),
                    )
                    nc.tensor.matmul(
                        pd[:psz, :], lhs, w4_sb[:, k, :],
                        start=(k == 0), stop=(k == HT - 1),
                    )
                lg = sp.tile([128, NCLS], F32, tag=f"lg{j}", name=f"lg{j}")
                nc.vector.tensor_add(lg[:psz, :], pl[:psz, :], lb_sb[:psz, :])
                nc.sync.dma_start(logits_d[p0:p0 + psz, :], lg[:psz, :])

                negmx = sp.tile([128, 1], F32, tag=f"nm{j}", name=f"nm{j}")
                nc.vector.reduce_max(negmx[:psz, :], lg[:psz, :], axis=AX, negate=True)
                ex = sp.tile([128, NCLS], F32, tag=f"ex{j}", name=f"ex{j}")
                sume = sp.tile([128, 1], F32, tag=f"se{j}", name=f"se{j}")
                nc.scalar.activation(
                    ex[:psz, :], lg[:psz, :], AF.Exp,
                    bias=negmx[:psz, 0:1], accum_out=sume[:psz, 0:1],
                )
                rcp = sp.tile([128, 1], F32, tag=f"rc{j}", name=f"rc{j}")
                nc.vector.reciprocal(rcp[:psz, :], sume[:psz, :])
                pr = sp.tile([128, NCLS], F32, tag=f"pr{j}", name=f"pr{j}")
                nc.vector.tensor_scalar_mul(pr[:psz, :], ex[:psz, :], rcp[:psz, 0:1])
                nc.sync.dma_start(probs_d[p0:p0 + psz, :], pr[:psz, :])

                dl = sp.tile([128, ND], F32, tag=f"dl{j}", name=f"dl{j}")
                nc.vector.tensor_add(dl[:psz, :], pd[:psz, :], db_sb[:psz, :])
                nc.sync.dma_start(deltas_d[p0:p0 + psz, :], dl[:psz, :])
    nc.compile()
    return nc


_CACHE = {}


def _compiled(which):
    if which not in _CACHE:
        _CACHE[which] = {"p1": build_p1, "p2": build_p2, "p3": build_p3}[which]()
    return _CACHE[which]


def _pack_inputs(inputs):
    g = {k: np.asarray(v) for k, v in inputs.items()}
    r16 = g["pooled_rois"].reshape(N_ROI, K1).astype(np.float16)
    w1 = np.ascontiguousarray(
        g["conv1_w"].reshape(K1, H).astype(np.float16)
        .reshape(K1T, 128, H).transpose(1, 0, 2)
    )
    w2 = np.ascontiguousarray(
        g["conv2_w"].reshape(H, H).astype(np.float16)
        .reshape(HT, 128, H).transpose(1, 0, 2)
    )
    w3 = np.ascontiguousarray(
        g["logits_w"].astype(np.float16).reshape(HT, 128, NCLS).transpose(1, 0, 2)
    )
    w4 = np.ascontiguousarray(
        g["delta_w"].astype(np.float16).reshape(HT, 128, ND).transpose(1, 0, 2)
    )
    rois = []
    for c in range(N_CORES):
        shard = r16[c * NROI_C:(c + 1) * NROI_C, :].T  # [12544, 250]
        rois.append(np.ascontiguousarray(
            shard.reshape(K1T, 128, NROI_C).transpose(1, 0, 2)
        ))
    return g, rois, w1, w2, w3, w4


def _bn_affine_host(stats_list, gamma, beta):
    """Combine per-core [128, 16] partial stats into per-channel affine
    (A = gamma*rstd, B = beta - mean*A), packed as [128, HT] f32."""
    tot = np.sum([s.astype(np.float32) for s in stats_list], axis=0)  # [128, 16]
    sums = tot[:, :HT].T.reshape(H)      # channel = m*128 + p
    sumsq = tot[:, HT:].T.reshape(H)
    mean = (sums / N_ROI).astype(np.float32)
    var = (sumsq / N_ROI).astype(np.float32) - mean * mean
    rstd = 1.0 / np.sqrt(var + np.float32(EPS))
    A = (gamma.astype(np.float32) * rstd).astype(np.float32)
    B = (beta.astype(np.float32) - mean * A).astype(np.float32)
    a_pk = np.ascontiguousarray(A.reshape(HT, 128).T)
    b_pk = np.ascontiguousarray(B.reshape(HT, 128).T)
    return a_pk, b_pk


def run_on_cores(inputs, trace=False, tmpdir=None, **kw):
    g, rois, w1, w2, w3, w4 = _pack_inputs(inputs)
    cores = list(range(N_CORES))

    res1 = run_bass_kernel_spmd(
        _compiled("p1"), [{"rois": rois[c], "w1": w1} for c in cores],
        core_ids=cores, trace=trace,
        tmpdir=(tmpdir + "_p1") if tmpdir else None, **kw,
    )
    a1, b1 = _bn_affine_host(
        [res1.results[c]["st"] for c in cores], g["bn1_gamma"], g["bn1_beta"]
    )
    res2 = run_bass_kernel_spmd(
        _compiled("p2"),
        [{"x1": res1.results[c]["x1"], "w2": w2, "a1": a1, "b1": b1} for c in cores],
        core_ids=cores, trace=trace,
        tmpdir=(tmpdir + "_p2") if tmpdir else None, **kw,
    )
    a2, b2 = _bn_affine_host(
        [res2.results[c]["st"] for c in cores], g["bn2_gamma"], g["bn2_beta"]
    )
    lb = np.ascontiguousarray(g["logits_b"].astype(np.float32))
    db = np.ascontiguousarray(g["delta_b"].astype(np.float32))
    res3 = run_bass_kernel_spmd(
        _compiled("p3"),
        [{"x2": res2.results[c]["x2"], "w3": w3, "w4": w4,
          "a2": a2, "b2": b2, "lb": lb, "db": db} for c in cores],
        core_ids=cores, trace=trace,
        tmpdir=(tmpdir + "_p3") if tmpdir else None, **kw,
    )
    logits = np.concatenate([res3.results[c]["logits"] for c in cores], axis=0)
    probs = np.concatenate([res3.results[c]["probs"] for c in cores], axis=0)
    deltas = np.concatenate([res3.results[c]["deltas"] for c in cores], axis=0)
    out = (
        logits.astype(np.float32),
        probs.astype(np.float32),
        deltas.reshape(N_ROI, NCLS, 4).astype(np.float32),
    )
    return out, (res1, res2, res3)


def kernel(**inputs):
    out, _ = run_on_cores(inputs)
    return out
